# revision 1
# baseline (speedup 1.0000x reference)
"""Biased self-attention TRN2 Bass kernel (8 NeuronCores).

Problem: nn_BiasedSelfAttention — B=2, N=2048, D=1024, H=16, DK=64.
    q,k,v = split_heads(x@Wq+bq), ...; k,v scaled by (1+alpha[b,n]);
    logits = q k^T/sqrt(DK) + bias[b][None]; y = softmax(logits) v;
    out = merge_heads(y) @ Wo + bo.

Sharding: 8 cores = (batch b in {0,1}) x (head-group hg in {0..3} of 4
heads = 256 dims of D).  Data parallel over B, tensor parallel over H.
Each core computes a partial O-projection (its 256 rows of Wo); the
host sums the 4 partials per batch (part of unsharding).

Device pipeline per core (all matmuls float32r = full-rate fp32):
  phase 1: Q,K projections with TRANSPOSED outputs [dk, n]; V natural
           [m, dk] with a ones column appended (softmax denominators).
           alpha folded in on host: K/V use xk = x*(1+alpha) as input;
           projection biases injected exactly as rank-1 K=1 matmuls.
  phase 2: per (n-chunk 512, m-tile 128): S^T = k^T-lhsT @ q^T-rhs
           (K=64, two heads packed on disjoint PE row-groups), DVE adds
           bias^T (host-pretransposed) from PSUM, ACT exp -> fp32r,
           AV matmuls accumulate y_aug^T = [v|1]^T E^T over m-tiles
           (row 64 = softmax denominator).  Normalize: DVE reciprocal,
           K=1 ones matmul broadcasts it over 64 partitions, DVE mul.
  phase 3: partial out = y^T-pair-lhsT @ Wo-rows + (1/4)bo rank-1.
"""

import json
import os
import sys

sys.path.insert(0, "/opt/trn_rl_repo")

import numpy as np

import concourse.bass as bass
import concourse.mybir as mybir
import concourse.tile as tile
from concourse.bass_utils import run_bass_kernel_spmd

# ---------------------------------------------------------------- bir fix --
# The pinned walrus encodes at most ONE sem-wait per instruction, but Tile's
# wait-assigner can emit several.  Hoist extras onto EventSemaphore
# instructions (what a standalone wait_ge lowers to) just before the
# instruction — waits gate dispatch at the engine sequencer, so this is
# semantically identical.


def _split_multi_waits(bir_json: bytes) -> bytes:
    m = json.loads(bir_json)
    n_split = 0
    for fn in m.get("functions", []):
        for blk in fn.get("blocks", []):
            insts = blk.get("instructions")
            if not insts:
                continue
            out = []
            for inst in insts:
                sync = inst.get("sync_info")
                waits = (sync or {}).get("on_wait") or []
                if len(waits) > 1:
                    for i, w in enumerate(waits[:-1]):
                        out.append({
                            "debug": inst.get("debug", 0),
                            "engine": inst["engine"],
                            "ins": [],
                            "name": f"{inst['name']}-sw{i}",
                            "opcode": "EventSemaphore",
                            "outs": [],
                            "sync_info": {"on_update": [], "on_wait": [w]},
                        })
                        n_split += 1
                    sync["on_wait"] = waits[-1:]
                out.append(inst)
            blk["instructions"] = out
    return json.dumps(m).encode()


def _patch_bass():
    if getattr(bass.Bass, "_multiwait_patched", False):
        return
    orig = bass.Bass.to_json_bytes

    def to_json_bytes(self, *a, **kw):
        return _split_multi_waits(orig(self, *a, **kw))

    bass.Bass.to_json_bytes = to_json_bytes
    bass.Bass._multiwait_patched = True


_patch_bass()

# ------------------------------------------------------------- dimensions --
B, N, D, H = 2, 2048, 1024, 16
DK = D // H                      # 64
NCORES = 8
HPC = H // 4                     # 4 heads per core
DSL = HPC * DK                   # 256 D-columns per core
NQ4 = N // 512                   # 4 query/key quarters
MT = N // 128                    # 16 key tiles
F32 = mybir.dt.float32
F32R = mybir.dt.float32r
Exp = mybir.ActivationFunctionType.Exp
Log = mybir.ActivationFunctionType.Ln
Copy = mybir.ActivationFunctionType.Copy


def _build_nc() -> bass.Bass:
    nc = bass.Bass()

    xT = nc.dram_tensor("xT", [D, N], F32R, kind="ExternalInput")
    xkT = nc.dram_tensor("xkT", [D, N], F32R, kind="ExternalInput")
    wq = nc.dram_tensor("wq", [D, DSL], F32R, kind="ExternalInput")
    wk = nc.dram_tensor("wk", [D, DSL], F32R, kind="ExternalInput")
    wv = nc.dram_tensor("wv", [D, DSL], F32R, kind="ExternalInput")
    wo = nc.dram_tensor("wo", [DSL, D], F32R, kind="ExternalInput")
    biasT = nc.dram_tensor("biasT", [N, N], F32R, kind="ExternalInput")
    bq_r = nc.dram_tensor("bq_r", [1, DSL], F32R, kind="ExternalInput")
    bk_r = nc.dram_tensor("bk_r", [1, DSL], F32R, kind="ExternalInput")
    bv_r = nc.dram_tensor("bv_r", [1, DSL], F32R, kind="ExternalInput")
    bo4 = nc.dram_tensor("bo4", [1, D], F32R, kind="ExternalInput")
    srow = nc.dram_tensor("srow", [1, N], F32R, kind="ExternalInput")
    onesrow = nc.dram_tensor("onesrow", [1, 512], F32R, kind="ExternalInput")
    onescol = nc.dram_tensor("onescol", [128, 1], F32R, kind="ExternalInput")
    ident = nc.dram_tensor("ident", [128, 128], F32R, kind="ExternalInput")
    out_part = nc.dram_tensor("out_part", [N, D], F32, kind="ExternalOutput")

    with tile.TileContext(nc) as tc:
        with tc.tile_pool(name="consts", bufs=1) as consts, \
             tc.tile_pool(name="persist", bufs=1) as persist, \
             tc.tile_pool(name="xin", bufs=2) as xin, \
             tc.tile_pool(name="stream", bufs=3) as stream, \
             tc.tile_pool(name="outp", bufs=2) as outp, \
             tc.tile_pool(name="work", bufs=2) as work, \
             tc.tile_pool(name="small", bufs=1) as small, \
             tc.tile_pool(name="psum", bufs=1, space="PSUM") as pp:

            # ---- constants -------------------------------------------------
            wq_t = consts.tile([128, 8, DSL], F32R, tag="wq")
            wk_t = consts.tile([128, 8, DSL], F32R, tag="wk")
            wv_t = consts.tile([128, 8, DSL], F32R, tag="wv")
            nc.sync.dma_start(out=wq_t, in_=wq.rearrange("(t p) j -> p t j", p=128))
            nc.sync.dma_start(out=wk_t, in_=wk.rearrange("(t p) j -> p t j", p=128))
            nc.sync.dma_start(out=wv_t, in_=wv.rearrange("(t p) j -> p t j", p=128))
            onescol_t = consts.tile([128, 1], F32R, tag="onescol")
            nc.sync.dma_start(out=onescol_t, in_=onescol[:])
            ident_t = consts.tile([128, 128], F32R, tag="ident")
            nc.sync.dma_start(out=ident_t, in_=ident[:])
            bq_t = consts.tile([1, DSL], F32R, tag="bq")
            bk_t = consts.tile([1, DSL], F32R, tag="bk")
            bv_t = consts.tile([1, DSL], F32R, tag="bv")
            bo4_t = consts.tile([1, D], F32R, tag="bo4")
            srow_t = consts.tile([1, N], F32R, tag="srow")
            ones_t = consts.tile([1, 512], F32R, tag="ones")
            nc.sync.dma_start(out=bq_t, in_=bq_r[:])
            nc.sync.dma_start(out=bk_t, in_=bk_r[:])
            nc.sync.dma_start(out=bv_t, in_=bv_r[:])
            nc.sync.dma_start(out=bo4_t, in_=bo4[:])
            nc.sync.dma_start(out=srow_t, in_=srow[:])
            nc.sync.dma_start(out=ones_t, in_=onesrow[:])

            # ---- persistent intermediates ---------------------------------
            # q^T/k^T head-pair tiles: [dk-pair row, hp, quarter, 512]
            qT_all = persist.tile([128, 2, 8, 256], F32R, tag="qT")
            kT_all = persist.tile([128, 2, 8, 256], F32R, tag="kT")
            # v natural + ones col: [m-part, m-tile, head, 65]
            vaug = persist.tile([128, MT, HPC, 65], F32R, tag="vaug")
            # y^T head-pair tiles for O-proj
            yT_all = persist.tile([128, 2, NQ4, 512], F32R, tag="yT")

            # ---- phase 1: projections -------------------------------------
            for q8 in range(8):
                sl = slice(q8 * 256, q8 * 256 + 256)
                xq = xin.tile([128, 8, 256], F32R, tag="xT")
                nc.sync.dma_start(
                    out=xq, in_=xT[:, sl].rearrange("(t p) n -> p t n", p=128))
                xkq = xin.tile([128, 8, 256], F32R, tag="xkT")
                nc.sync.dma_start(
                    out=xkq, in_=xkT[:, sl].rearrange("(t p) n -> p t n", p=128))

                for w_t, rhs_t, inj_b, inj_r, inj_rsl, scale, dest in (
                    (wq_t, xq, bq_t, ones_t, slice(0, 256), 0.125, qT_all),
                    (wk_t, xkq, bk_t, srow_t, sl, 1.0, kT_all),
                ):
                    ps = pp.tile([128, 2, 256], F32, tag="s", bufs=4)
                    for hp in range(2):
                        csl = slice(hp * 128, hp * 128 + 128)
                        for t in range(8):
                            nc.tensor.matmul(
                                ps[:, hp], w_t[:, t, csl], rhs_t[:, t, :],
                                start=(t == 0), stop=False)
                        nc.tensor.matmul(
                            ps[:, hp], inj_b[0:1, csl], inj_r[0:1, inj_rsl],
                            start=False, stop=True)
                    nc.scalar.activation(dest[:, :, q8, :], ps, Copy, scale=scale)

                for j in range(2):
                    mt = q8 * 2 + j
                    msl = slice(j * 128, j * 128 + 128)
                    ps = pp.tile([128, 256], F32, tag="y", bufs=1)
                    for t in range(8):
                        nc.tensor.matmul(
                            ps, xkq[:, t, msl], wv_t[:, t, :],
                            start=(t == 0), stop=False)
                    nc.tensor.matmul(
                        ps, srow_t[0:1, mt * 128:mt * 128 + 128], bv_t[0:1, :],
                        start=False, stop=True)
                    nc.scalar.activation(
                        vaug[:, mt, :, 0:64],
                        ps.rearrange("p (h d) -> p h d", h=HPC), Copy)
                    nc.vector.tensor_copy(
                        vaug[:, mt, :, 64:65],
                        onescol_t.unsqueeze(1).broadcast_to([128, HPC, 1]))

            # ---- phase 2 + 3, software-pipelined across quarters ----------
            # Per n-quarter: 32 rounds of (QK-pair matmuls -> bias add (DVE or
            # PE-inject) -> ACT exp -> AV accumulate).  The normalize tail of
            # quarter q and its 4 O-projection tiles are emitted EARLY inside
            # quarter q+1's round stream so the PE never idles long enough for
            # HAM to re-throttle.
            wo_t = consts.tile([128, 2, D], F32R, tag="wo")
            nc.sync.dma_start(out=wo_t, in_=wo.rearrange("(t p) j -> p t j", p=128))

            n_rounds = MT * 2
            state = {}

            def qk_round(q4, r):
                nsl = slice(q4 * 512, q4 * 512 + 512)
                mt, rr = divmod(r, 2)
                if rr == 0:
                    b_t = stream.tile([128, 512], F32R, tag="bias")
                    nc.sync.dma_start(
                        out=b_t, in_=biasT[mt * 128:mt * 128 + 128, nsl])
                    state["b_cur"] = b_t
                b_t = state["b_cur"]
                s_list = []
                for hi in range(2):
                    h = rr * 2 + hi
                    hp = h // 2
                    rsl = slice((h % 2) * 64, (h % 2) * 64 + 64)
                    s_ps = pp.tile([128, 512], F32, tag="s", bufs=4,
                                   name=f"s{r}_{hi}")
                    nc.tensor.matmul(
                        s_ps,
                        kT_all[rsl, hp, mt // 2,
                               (mt % 2) * 128:(mt % 2) * 128 + 128],
                        qT_all[rsl, hp, 2 * q4:2 * q4 + 2, :],
                        start=True, stop=False)
                    s_list.append(s_ps)
                e_list = []
                for hi in range(2):
                    # PE adds the (host-pretransposed) bias via identity matmul
                    nc.tensor.matmul(
                        s_list[hi], ident_t, b_t, start=False, stop=True)
                    e_t = work.tile([128, 512], F32R, tag="e", bufs=6,
                                    name=f"e{r}_{hi}")
                    nc.scalar.activation(e_t, s_list[hi], Exp)
                    e_list.append(e_t)
                state[("e", r % 3)] = e_list

            def av_round(q4, r):
                mt, rr = divmod(r, 2)
                e_list = state[("e", r % 3)]
                y_ps = state[("y", q4)]
                for hi in range(2):
                    h = rr * 2 + hi
                    nc.tensor.matmul(
                        y_ps[:, h], vaug[:, mt, h, :], e_list[hi],
                        start=(mt == 0), stop=(mt == MT - 1))

            def tail(q4):
                y_ps = state.pop(("y", q4))
                y_sb = small.tile([65, HPC, 512], F32, tag="ysb")
                nc.scalar.activation(y_sb, y_ps, Copy)
                # r = 1/D via exp(-ln(D)) on ACT (shares one table set with
                # the main exp; keeps the single-lane recip off the DVE)
                lnD = small.tile([1, HPC, 512], F32, tag="lnD")
                nc.scalar.activation(lnD, y_sb[64:65, :, :], Log)
                r_row = small.tile([1, HPC, 512], F32R, tag="r")
                nc.scalar.activation(r_row, lnD, Exp, scale=-1.0)
                with nc.allow_low_precision(reason="y fp32r for O-proj"):
                    for h in range(HPC):
                        rb_ps = pp.tile([64, 512], F32, tag="s", bufs=4,
                                        name=f"rb{q4}_{h}")
                        nc.tensor.matmul(
                            rb_ps, ones_t[0:1, 0:64],
                            r_row[:, h, :], start=True, stop=True)
                        hp, hi2 = divmod(h, 2)
                        nc.vector.tensor_mul(
                            yT_all[hi2 * 64:hi2 * 64 + 64, hp, q4, :],
                            y_sb[0:64, h, :], rb_ps)

            def oproj(q4, j):
                nt = q4 * 4 + j
                o_sb = outp.tile([128, D], F32, tag="osb")
                for dc in range(2):
                    o_ps = pp.tile([128, 512], F32, tag="s", bufs=4,
                                   name=f"o{nt}_{dc}")
                    for hp in range(2):
                        nc.tensor.matmul(
                            o_ps,
                            yT_all[:, hp, q4, j * 128:j * 128 + 128],
                            wo_t[:, hp, dc * 512:dc * 512 + 512],
                            start=(hp == 0), stop=False)
                    nc.tensor.matmul(
                        o_ps, ones_t[0:1, 0:128],
                        bo4_t[0:1, dc * 512:dc * 512 + 512],
                        start=False, stop=True)
                    nc.scalar.activation(
                        o_sb[:, dc * 512:dc * 512 + 512], o_ps, Copy)
                nc.sync.dma_start(
                    out=out_part[nt * 128:nt * 128 + 128, :], in_=o_sb)

            # flattened pipeline over quarters
            for q4 in range(NQ4):
                state[("y", q4)] = pp.tile([65, HPC, 512], F32, tag="y", bufs=1, name=f"y_ps{q4}")
                qk_round(q4, 0)
                for r in range(1, n_rounds):
                    qk_round(q4, r)
                    av_round(q4, r - 1)
                    if q4 > 0:
                        # interleave previous quarter's tail + O-proj early
                        if r == 2:
                            tail(q4 - 1)
                        elif 3 <= r <= 6:
                            oproj(q4 - 1, r - 3)
                av_round(q4, n_rounds - 1)
            tail(NQ4 - 1)
            for j in range(4):
                oproj(NQ4 - 1, j)

    return nc


def _ensure_ntff_hook():
    """Register the axon NTFF profiling hook if the agent image lacks
    antenv.axon_hooks (profiling only; kernel runs fine without)."""
    try:
        from antenv.axon_hooks import get_axon_ntff_profile_hook  # noqa: F401
        return
    except ImportError:
        pass
    import types
    import antenv
    from trn_agent_boot.trn_boot import _ntff_profile_via_ctypes

    mod = types.ModuleType("antenv.axon_hooks")
    holder = {}
    mod.set_axon_ntff_profile_hook = lambda h: holder.__setitem__("h", h)
    mod.get_axon_ntff_profile_hook = lambda: holder.get("h")
    sys.modules["antenv.axon_hooks"] = mod
    antenv.axon_hooks = mod
    mod.set_axon_ntff_profile_hook(
        _ntff_profile_via_ctypes("/opt/axon/libaxon_pjrt.so"))


_NC_CACHE: dict = {}


def _get_nc() -> bass.Bass:
    if "nc" not in _NC_CACHE:
        _NC_CACHE["nc"] = _build_nc()
    return _NC_CACHE["nc"]


def kernel(x, alpha, bias, Wq, bq, Wk, bk, Wv, bv, Wo, bo, trace=False):
    x = np.asarray(x, np.float32)
    alpha = np.asarray(alpha, np.float32)
    bias = np.asarray(bias, np.float32)
    Wq = np.asarray(Wq, np.float32); bq = np.asarray(bq, np.float32)
    Wk = np.asarray(Wk, np.float32); bk = np.asarray(bk, np.float32)
    Wv = np.asarray(Wv, np.float32); bv = np.asarray(bv, np.float32)
    Wo = np.asarray(Wo, np.float32); bo = np.asarray(bo, np.float32)

    c = np.ascontiguousarray
    onescol = np.ones((128, 1), np.float32)
    onesrow = np.ones((1, 512), np.float32)
    bo4 = (bo / 4.0).reshape(1, D)

    in_maps = []
    per_b = {}
    for b in range(B):
        s = 1.0 + alpha[b]                        # (N,)
        per_b[b] = {
            "xT": c(x[b].T),                      # (D, N)
            "xkT": c((x[b] * s[:, None]).T),      # (D, N)
            "biasT": c(bias[b].T),                # (N, N)  [m, n]
            "srow": s.reshape(1, N),
        }
    for core in range(NCORES):
        b, hg = divmod(core, 4)
        dsl = slice(hg * DSL, hg * DSL + DSL)
        in_maps.append({
            **per_b[b],
            "wq": c(Wq[:, dsl]), "wk": c(Wk[:, dsl]), "wv": c(Wv[:, dsl]),
            "wo": c(Wo[dsl, :]),
            "bq_r": c(bq[dsl].reshape(1, DSL)),
            "bk_r": c(bk[dsl].reshape(1, DSL)),
            "bv_r": c(bv[dsl].reshape(1, DSL)),
            "bo4": bo4,
            "onesrow": onesrow, "onescol": onescol,
            "ident": np.eye(128, dtype=np.float32),
        })

    if trace:
        _ensure_ntff_hook()
    nc = _get_nc()
    res = run_bass_kernel_spmd(
        nc, in_maps, core_ids=list(range(NCORES)), trace=trace)

    out = np.zeros((B, N, D), np.float32)
    for core in range(NCORES):
        out[core // 4] += res.results[core]["out_part"]
    if trace:
        kernel.last_exec_time_ns = res.exec_time_ns
        kernel.last_profile = res.profile_json
    return out



# revision 5
# speedup vs baseline: 1.0742x; 1.0742x over previous
"""Biased self-attention TRN2 Bass kernel (8 NeuronCores), v2.

Problem: nn_BiasedSelfAttention — B=2, N=2048, D=1024, H=16, DK=64.
    q,k,v = split_heads(x@Wq+bq), ...; k,v scaled by (1+alpha[b,n]);
    logits = q k^T/sqrt(DK) + bias[b][None]; y = softmax(logits) v;
    out = merge_heads(y) @ Wo + bo.

Sharding: 8 cores = (batch b in {0,1}) x (head-group hg in {0..3} of 4
heads = 256 dims of D).  Data parallel over B, tensor parallel over H.
Each core computes a partial O-projection (its 256 rows of Wo); the
host sums the 4 partials per batch and adds bo.

v2 design (from NTFF profile of v1: PE 88% busy incl. 4.3 GF of bias
identity-injects, ACT 53%, DVE 3%, 33% HAM-throttled):
  - all matmul operands bf16 (same PE rate as f32r, half the DMA/SBUF)
  - bias add alternates per-round between PE identity-inject (into
    PSUM) and DVE tensor_tensor add (PSUM+SBUF->SBUF), balancing the
    three engines; exp is one ACT instr per round over the head-pair
  - projection phase: K then V then Q, x/xk fully resident in SBUF
  - normalize tail: ones-row K=1 matmul broadcasts denominators,
    DVE reciprocal_approx_fast + mul; bo added on host
"""

import json
import sys

sys.path.insert(0, "/opt/trn_rl_repo")

import numpy as np
import ml_dtypes

import concourse.bass as bass
import concourse.mybir as mybir
import concourse.tile as tile
from concourse.bass_utils import run_bass_kernel_spmd

# ---------------------------------------------------------------- bir fix --
# The pinned walrus encodes at most ONE sem-wait per instruction, but Tile's
# wait-assigner can emit several.  Hoist extras onto EventSemaphore
# instructions just before the instruction.


def _split_multi_waits(bir_json: bytes) -> bytes:
    m = json.loads(bir_json)
    for fn in m.get("functions", []):
        for blk in fn.get("blocks", []):
            insts = blk.get("instructions")
            if not insts:
                continue
            out = []
            for inst in insts:
                sync = inst.get("sync_info")
                waits = (sync or {}).get("on_wait") or []
                if len(waits) > 1:
                    for i, w in enumerate(waits[:-1]):
                        out.append({
                            "debug": inst.get("debug", 0),
                            "engine": inst["engine"],
                            "ins": [],
                            "name": f"{inst['name']}-sw{i}",
                            "opcode": "EventSemaphore",
                            "outs": [],
                            "sync_info": {"on_update": [], "on_wait": [w]},
                        })
                    sync["on_wait"] = waits[-1:]
                out.append(inst)
            blk["instructions"] = out
    return json.dumps(m).encode()


def _patch_bass():
    if getattr(bass.Bass, "_multiwait_patched", False):
        return
    orig = bass.Bass.to_json_bytes

    def to_json_bytes(self, *a, **kw):
        return _split_multi_waits(orig(self, *a, **kw))

    bass.Bass.to_json_bytes = to_json_bytes
    bass.Bass._multiwait_patched = True


_patch_bass()

# ------------------------------------------------------------- dimensions --
B, N, D, H = 2, 2048, 1024, 16
DK = D // H                      # 64
NCORES = 8
HPC = H // 4                     # 4 heads per core
DSL = HPC * DK                   # 256 D-columns per core
NQ4 = N // 512                   # 4 query quarters
MT = N // 128                    # 16 key tiles
F32 = mybir.dt.float32
F32R = mybir.dt.float32r
BF16 = mybir.dt.bfloat16
Exp = mybir.ActivationFunctionType.Exp
Copy = mybir.ActivationFunctionType.Copy
Add = mybir.AluOpType.add

# round flavor: True -> bias injected on PE; False -> bias added on DVE.
# Alternate to balance PE (~852ns/round) vs DVE (~658ns/round) vs ACT
# (~1038ns/round exp).
PE_BIAS_ROUND = [r % 2 == 0 for r in range(MT * 2)]


def _build_nc() -> bass.Bass:
    nc = bass.Bass()

    xT = nc.dram_tensor("xT", [D, N], BF16, kind="ExternalInput")
    xkT = nc.dram_tensor("xkT", [D, N], BF16, kind="ExternalInput")
    wq = nc.dram_tensor("wq", [D, DSL], BF16, kind="ExternalInput")
    wk = nc.dram_tensor("wk", [D, DSL], BF16, kind="ExternalInput")
    wv = nc.dram_tensor("wv", [D, DSL], BF16, kind="ExternalInput")
    wo = nc.dram_tensor("wo", [DSL, D], BF16, kind="ExternalInput")
    biasT = nc.dram_tensor("biasT", [N, N], BF16, kind="ExternalInput")
    bq_r = nc.dram_tensor("bq_r", [1, DSL], BF16, kind="ExternalInput")
    bk_r = nc.dram_tensor("bk_r", [1, DSL], BF16, kind="ExternalInput")
    bv_r = nc.dram_tensor("bv_r", [1, DSL], BF16, kind="ExternalInput")
    srow = nc.dram_tensor("srow", [1, N], BF16, kind="ExternalInput")
    onesrow = nc.dram_tensor("onesrow", [1, 512], BF16, kind="ExternalInput")
    ones64 = nc.dram_tensor("ones64", [65, 64], F32R, kind="ExternalInput")
    onescol = nc.dram_tensor("onescol", [128, 1], BF16, kind="ExternalInput")
    identb = nc.dram_tensor("identb", [128, 128], BF16, kind="ExternalInput")
    out_part = nc.dram_tensor("out_part", [N, D], F32, kind="ExternalOutput")

    with tile.TileContext(nc) as tc:
        with tc.tile_pool(name="consts", bufs=1) as consts, \
             tc.tile_pool(name="persist", bufs=1) as persist, \
             tc.tile_pool(name="stream", bufs=4) as stream, \
             tc.tile_pool(name="work", bufs=3) as work, \
             tc.tile_pool(name="outp", bufs=2) as outp, \
             tc.tile_pool(name="psum", bufs=1, space="PSUM") as pp:

            # ---- constants -------------------------------------------------
            xT_sb = consts.tile([128, 8, N], BF16, tag="xT")
            xkT_sb = consts.tile([128, 8, N], BF16, tag="xkT")
            nc.sync.dma_start(out=xT_sb, in_=xT.rearrange("(t p) n -> p t n", p=128))
            nc.sync.dma_start(out=xkT_sb, in_=xkT.rearrange("(t p) n -> p t n", p=128))
            wq_t = consts.tile([128, 8, DSL], BF16, tag="wq")
            wk_t = consts.tile([128, 8, DSL], BF16, tag="wk")
            wv_t = consts.tile([128, 8, DSL], BF16, tag="wv")
            nc.sync.dma_start(out=wq_t, in_=wq.rearrange("(t p) j -> p t j", p=128))
            nc.sync.dma_start(out=wk_t, in_=wk.rearrange("(t p) j -> p t j", p=128))
            nc.sync.dma_start(out=wv_t, in_=wv.rearrange("(t p) j -> p t j", p=128))
            wo_t = consts.tile([128, 2, D], BF16, tag="wo")
            nc.sync.dma_start(out=wo_t, in_=wo.rearrange("(t p) j -> p t j", p=128))
            identb_t = consts.tile([128, 128], BF16, tag="identb")
            nc.sync.dma_start(out=identb_t, in_=identb[:])
            bq_t = consts.tile([1, DSL], BF16, tag="bq")
            bk_t = consts.tile([1, DSL], BF16, tag="bk")
            bv_t = consts.tile([1, DSL], BF16, tag="bv")
            srow_t = consts.tile([1, N], BF16, tag="srow")
            ones_t = consts.tile([1, 512], BF16, tag="ones")
            ones64_t = consts.tile([65, 64], F32R, tag="ones64")
            onescol_t = consts.tile([128, 1], BF16, tag="onescol")
            nc.sync.dma_start(out=bq_t, in_=bq_r[:])
            nc.sync.dma_start(out=bk_t, in_=bk_r[:])
            nc.sync.dma_start(out=bv_t, in_=bv_r[:])
            nc.sync.dma_start(out=srow_t, in_=srow[:])
            nc.sync.dma_start(out=ones_t, in_=onesrow[:])
            nc.sync.dma_start(out=ones64_t, in_=ones64[:])
            nc.sync.dma_start(out=onescol_t, in_=onescol[:])

            # ---- persistent intermediates ---------------------------------
            # q^T/k^T: [dk-pair row (h%2)*64+dk, hp, n]
            qT_all = persist.tile([128, 2, N], BF16, tag="qT")
            kT_all = persist.tile([128, 2, N], BF16, tag="kT")
            # v natural + ones col: [m-part, m-tile, head, 65]
            vaug = persist.tile([128, MT, HPC, 65], BF16, tag="vaug")
            # normalized y^T for O-proj
            yT_all = persist.tile([128, 2, N], BF16, tag="yT")
            # per-quarter y + denominators staging
            y_sb = persist.tile([65, HPC, 512], F32R, tag="ysb")

            # vaug ones columns, written once
            nc.vector.tensor_copy(
                vaug[:, :, :, 64:65],
                onescol_t.unsqueeze(1).unsqueeze(1).broadcast_to([128, MT, HPC, 1]))

            # ---- phase 1: projections (K all, V all, Q all) ---------------
            # K chunks
            for c in range(4):
                nsl = slice(c * 512, c * 512 + 512)
                ps = pp.tile([128, 2, 512], F32, tag="s", bufs=2, name=f"kps{c}")
                for hp in range(2):
                    csl = slice(hp * 128, hp * 128 + 128)
                    for t in range(8):
                        nc.tensor.matmul(
                            ps[:, hp], wk_t[:, t, csl], xkT_sb[:, t, nsl],
                            start=(t == 0), stop=False)
                    nc.tensor.matmul(
                        ps[:, hp], bk_t[0:1, csl], srow_t[0:1, nsl],
                        start=False, stop=True)
                nc.vector.tensor_copy(kT_all[:, :, nsl], ps)

            # V m-tiles (PSUM staged in the y-tag slot, 4 subtiles ping-pong)
            vps = pp.tile([128, 4, 256], F32, tag="y", bufs=1, name="vps")
            for mt in range(MT):
                msl = slice(mt * 128, mt * 128 + 128)
                vp = vps[:, mt % 4, :]
                for t in range(8):
                    nc.tensor.matmul(
                        vp, xkT_sb[:, t, msl], wv_t[:, t, :],
                        start=(t == 0), stop=False)
                nc.tensor.matmul(
                    vp, srow_t[0:1, msl], bv_t[0:1, :], start=False, stop=True)
                vr = vp.rearrange("p (h d) -> p h d", h=HPC)
                if mt % 2 == 0:
                    nc.vector.tensor_copy(vaug[:, mt, :, 0:64], vr)
                else:
                    nc.scalar.activation(vaug[:, mt, :, 0:64], vr, Copy)

            # Q chunks (scale 1/sqrt(DK)=0.125 folded into the copy)
            for c in range(4):
                nsl = slice(c * 512, c * 512 + 512)
                ps = pp.tile([128, 2, 512], F32, tag="s", bufs=2, name=f"qps{c}")
                for hp in range(2):
                    csl = slice(hp * 128, hp * 128 + 128)
                    for t in range(8):
                        nc.tensor.matmul(
                            ps[:, hp], wq_t[:, t, csl], xT_sb[:, t, nsl],
                            start=(t == 0), stop=False)
                    nc.tensor.matmul(
                        ps[:, hp], bq_t[0:1, csl], ones_t[0:1, :],
                        start=False, stop=True)
                nc.vector.tensor_scalar_mul(qT_all[:, :, nsl], ps, 0.125)

            # ---- phase 2+3, software-pipelined across quarters ------------
            n_rounds = MT * 2
            state = {}

            def qk_round(q4, r):
                nsl = slice(q4 * 512, q4 * 512 + 512)
                mt, rr = divmod(r, 2)
                pe_bias = PE_BIAS_ROUND[r]
                if rr == 0:
                    b_t = stream.tile([128, 512], BF16, tag="bias",
                                      name=f"b{q4}_{mt}")
                    nc.sync.dma_start(
                        out=b_t, in_=biasT[mt * 128:mt * 128 + 128, nsl])
                    state["b_cur"] = b_t
                b_t = state["b_cur"]
                s_ps = pp.tile([128, 2, 512], F32, tag="s", bufs=2,
                               name=f"s{q4}_{r}")
                for hi in range(2):
                    h = rr * 2 + hi
                    hp = h // 2
                    rsl = slice((h % 2) * 64, (h % 2) * 64 + 64)
                    nc.tensor.matmul(
                        s_ps[:, hi],
                        kT_all[rsl, hp, mt * 128:mt * 128 + 128],
                        qT_all[rsl, hp, nsl],
                        start=True, stop=(not pe_bias))
                e_t = work.tile([128, 2, 512], BF16, tag="e", bufs=3,
                                name=f"e{q4}_{r}")
                if pe_bias:
                    for hi in range(2):
                        nc.tensor.matmul(
                            s_ps[:, hi], identb_t, b_t, start=False, stop=True)
                    nc.scalar.activation(e_t, s_ps, Exp)
                else:
                    sb_s = work.tile([128, 2, 512], F32, tag="sbs", bufs=2,
                                     name=f"sb{q4}_{r}")
                    for hi in range(2):
                        nc.vector.tensor_tensor(
                            sb_s[:, hi], s_ps[:, hi], b_t, Add)
                    nc.scalar.activation(e_t, sb_s, Exp)
                state[("e", r % 3)] = e_t

            def av_round(q4, r):
                mt, rr = divmod(r, 2)
                e_t = state[("e", r % 3)]
                y_ps = state[("y", q4)]
                for hi in range(2):
                    h = rr * 2 + hi
                    nc.tensor.matmul(
                        y_ps[:, h], vaug[:, mt, h, :], e_t[:, hi],
                        start=(mt == 0), stop=(mt == MT - 1))

            def tail(q4):
                y_ps = state.pop(("y", q4))
                nc.vector.tensor_copy(y_sb, y_ps)
                for h in range(HPC):
                    rb = pp.tile([128, 2, 512], F32, tag="s", bufs=2,
                                 name=f"rb{q4}_{h}")
                    nc.tensor.matmul(
                        rb[0:64, 0, :], ones64_t[64:65, :], y_sb[64:65, h, :],
                        start=True, stop=True)
                    rcp = work.tile([64, 512], F32, tag="rcp", bufs=2,
                                    name=f"rcp{q4}_{h}")
                    nc.vector.reciprocal(out=rcp, in_=rb[0:64, 0, :])
                    hp, hi2 = divmod(h, 2)
                    nc.vector.tensor_mul(
                        yT_all[hi2 * 64:hi2 * 64 + 64, hp,
                               q4 * 512:q4 * 512 + 512],
                        y_sb[0:64, h, :].bitcast(F32), rcp)

            def oproj(q4, j):
                nt = q4 * 4 + j
                o_ps = pp.tile([128, 2, 512], F32, tag="s", bufs=2,
                               name=f"o{nt}")
                for dc in range(2):
                    for hp in range(2):
                        nc.tensor.matmul(
                            o_ps[:, dc],
                            yT_all[:, hp, nt * 128:nt * 128 + 128],
                            wo_t[:, hp, dc * 512:dc * 512 + 512],
                            start=(hp == 0), stop=(hp == 1))
                o_sb = outp.tile([128, D], F32, tag="osb", name=f"ob{nt}")
                if j % 2 == 0:
                    nc.scalar.activation(o_sb.rearrange("p (c f) -> p c f", c=2),
                                         o_ps, Copy)
                else:
                    nc.vector.tensor_copy(o_sb.rearrange("p (c f) -> p c f", c=2),
                                          o_ps)
                nc.sync.dma_start(
                    out=out_part[nt * 128:nt * 128 + 128, :], in_=o_sb)

            for q4 in range(NQ4):
                state[("y", q4)] = pp.tile(
                    [65, HPC, 512], F32, tag="y", bufs=1, name=f"y_ps{q4}")
                qk_round(q4, 0)
                for r in range(1, n_rounds):
                    qk_round(q4, r)
                    av_round(q4, r - 1)
                    if q4 > 0:
                        # previous quarter's tail + O-proj, interleaved early
                        if r == 2:
                            tail(q4 - 1)
                        elif r in (4, 8, 12, 16):
                            oproj(q4 - 1, (r - 4) // 4)
                av_round(q4, n_rounds - 1)
            tail(NQ4 - 1)
            for j in range(4):
                oproj(NQ4 - 1, j)

    return nc


def _ensure_ntff_hook():
    """Register the axon NTFF profiling hook if the agent image lacks
    antenv.axon_hooks (profiling only; kernel runs fine without)."""
    try:
        from antenv.axon_hooks import get_axon_ntff_profile_hook  # noqa: F401
        return
    except ImportError:
        pass
    import types
    import antenv
    from trn_agent_boot.trn_boot import _ntff_profile_via_ctypes

    mod = types.ModuleType("antenv.axon_hooks")
    holder = {}
    mod.set_axon_ntff_profile_hook = lambda h: holder.__setitem__("h", h)
    mod.get_axon_ntff_profile_hook = lambda: holder.get("h")
    sys.modules["antenv.axon_hooks"] = mod
    antenv.axon_hooks = mod
    mod.set_axon_ntff_profile_hook(
        _ntff_profile_via_ctypes("/opt/axon/libaxon_pjrt.so"))


_NC_CACHE: dict = {}


def _get_nc() -> bass.Bass:
    if "nc" not in _NC_CACHE:
        _NC_CACHE["nc"] = _build_nc()
    return _NC_CACHE["nc"]


def kernel(x, alpha, bias, Wq, bq, Wk, bk, Wv, bv, Wo, bo, trace=False):
    bf = ml_dtypes.bfloat16
    x = np.asarray(x, np.float32)
    alpha = np.asarray(alpha, np.float32)
    bias = np.asarray(bias, np.float32)
    Wq = np.asarray(Wq, np.float32); bq = np.asarray(bq, np.float32)
    Wk = np.asarray(Wk, np.float32); bk = np.asarray(bk, np.float32)
    Wv = np.asarray(Wv, np.float32); bv = np.asarray(bv, np.float32)
    Wo = np.asarray(Wo, np.float32); bo = np.asarray(bo, np.float32)

    c = np.ascontiguousarray

    in_maps = []
    per_b = {}
    for b in range(B):
        s = 1.0 + alpha[b]                             # (N,)
        per_b[b] = {
            "xT": c(x[b].T.astype(bf)),                # (D, N)
            "xkT": c((x[b] * s[:, None]).T.astype(bf)),
            "biasT": c(bias[b].T.astype(bf)),          # (N, N) [m, n]
            "srow": s.reshape(1, N).astype(bf),
        }
    for core in range(NCORES):
        b, hg = divmod(core, 4)
        dsl = slice(hg * DSL, hg * DSL + DSL)
        in_maps.append({
            **per_b[b],
            "wq": c(Wq[:, dsl].astype(bf)),
            "wk": c(Wk[:, dsl].astype(bf)),
            "wv": c(Wv[:, dsl].astype(bf)),
            "wo": c(Wo[dsl, :].astype(bf)),
            "bq_r": c(bq[dsl].reshape(1, DSL).astype(bf)),
            "bk_r": c(bk[dsl].reshape(1, DSL).astype(bf)),
            "bv_r": c(bv[dsl].reshape(1, DSL).astype(bf)),
            "onesrow": np.ones((1, 512), bf),
            "ones64": np.ones((65, 64), np.float32),
            "onescol": np.ones((128, 1), bf),
            "identb": np.eye(128, dtype=bf),
        })

    if trace:
        _ensure_ntff_hook()
    nc = _get_nc()
    res = run_bass_kernel_spmd(
        nc, in_maps, core_ids=list(range(NCORES)), trace=trace)

    out = np.zeros((B, N, D), np.float32)
    for core in range(NCORES):
        out[core // 4] += res.results[core]["out_part"]
    out += bo[None, None, :]
    if trace:
        kernel.last_exec_time_ns = res.exec_time_ns
        kernel.last_profile = res.profile_json
    return out


# revision 6
# speedup vs baseline: 1.4297x; 1.3309x over previous
"""Biased self-attention TRN2 Bass kernel (8 NeuronCores), v2.

Problem: nn_BiasedSelfAttention — B=2, N=2048, D=1024, H=16, DK=64.
    q,k,v = split_heads(x@Wq+bq), ...; k,v scaled by (1+alpha[b,n]);
    logits = q k^T/sqrt(DK) + bias[b][None]; y = softmax(logits) v;
    out = merge_heads(y) @ Wo + bo.

Sharding: 8 cores = (batch b in {0,1}) x (head-group hg in {0..3} of 4
heads = 256 dims of D).  Data parallel over B, tensor parallel over H.
Each core computes a partial O-projection (its 256 rows of Wo); the
host sums the 4 partials per batch and adds bo.

v2 design (from NTFF profile of v1: PE 88% busy incl. 4.3 GF of bias
identity-injects, ACT 53%, DVE 3%, 33% HAM-throttled):
  - all matmul operands bf16 (same PE rate as f32r, half the DMA/SBUF)
  - bias add alternates per-round between PE identity-inject (into
    PSUM) and DVE tensor_tensor add (PSUM+SBUF->SBUF), balancing the
    three engines; exp is one ACT instr per round over the head-pair
  - projection phase: K then V then Q, x/xk fully resident in SBUF
  - normalize tail: ones-row K=1 matmul broadcasts denominators,
    DVE reciprocal_approx_fast + mul; bo added on host
"""

import json
import sys

sys.path.insert(0, "/opt/trn_rl_repo")

import numpy as np
import ml_dtypes

import concourse.bass as bass
import concourse.mybir as mybir
import concourse.tile as tile
from concourse.bass_utils import run_bass_kernel_spmd

# ---------------------------------------------------------------- bir fix --
# The pinned walrus encodes at most ONE sem-wait per instruction, but Tile's
# wait-assigner can emit several.  Hoist extras onto EventSemaphore
# instructions just before the instruction.


def _split_multi_waits(bir_json: bytes) -> bytes:
    m = json.loads(bir_json)
    for fn in m.get("functions", []):
        for blk in fn.get("blocks", []):
            insts = blk.get("instructions")
            if not insts:
                continue
            out = []
            for inst in insts:
                sync = inst.get("sync_info")
                waits = (sync or {}).get("on_wait") or []
                if len(waits) > 1:
                    for i, w in enumerate(waits[:-1]):
                        out.append({
                            "debug": inst.get("debug", 0),
                            "engine": inst["engine"],
                            "ins": [],
                            "name": f"{inst['name']}-sw{i}",
                            "opcode": "EventSemaphore",
                            "outs": [],
                            "sync_info": {"on_update": [], "on_wait": [w]},
                        })
                    sync["on_wait"] = waits[-1:]
                out.append(inst)
            blk["instructions"] = out
    return json.dumps(m).encode()


def _patch_bass():
    if getattr(bass.Bass, "_multiwait_patched", False):
        return
    orig = bass.Bass.to_json_bytes

    def to_json_bytes(self, *a, **kw):
        return _split_multi_waits(orig(self, *a, **kw))

    bass.Bass.to_json_bytes = to_json_bytes
    bass.Bass._multiwait_patched = True


_patch_bass()

# ------------------------------------------------------------- dimensions --
B, N, D, H = 2, 2048, 1024, 16
DK = D // H                      # 64
NCORES = 8
HPC = H // 4                     # 4 heads per core
DSL = HPC * DK                   # 256 D-columns per core
NQ4 = N // 512                   # 4 query quarters
MT = N // 128                    # 16 key tiles
F32 = mybir.dt.float32
F32R = mybir.dt.float32r
BF16 = mybir.dt.bfloat16
Exp = mybir.ActivationFunctionType.Exp
Copy = mybir.ActivationFunctionType.Copy
Add = mybir.AluOpType.add

# round flavor: True -> bias injected on PE; False -> bias added on DVE.
# Alternate to balance PE (~852ns/round) vs DVE (~658ns/round) vs ACT
# (~1038ns/round exp).
PE_BIAS_ROUND = [(r % 2 == 0) or (r >= 28) for r in range(MT * 2)]


def _build_nc() -> bass.Bass:
    nc = bass.Bass()

    xT = nc.dram_tensor("xT", [D, N], BF16, kind="ExternalInput")
    xkT = nc.dram_tensor("xkT", [D, N], BF16, kind="ExternalInput")
    wq = nc.dram_tensor("wq", [D, DSL], BF16, kind="ExternalInput")
    wk = nc.dram_tensor("wk", [D, DSL], BF16, kind="ExternalInput")
    wv = nc.dram_tensor("wv", [D, DSL], BF16, kind="ExternalInput")
    wo = nc.dram_tensor("wo", [DSL, D], BF16, kind="ExternalInput")
    biasT = nc.dram_tensor("biasT", [N, N], BF16, kind="ExternalInput")
    bq_r = nc.dram_tensor("bq_r", [1, DSL], BF16, kind="ExternalInput")
    bk_r = nc.dram_tensor("bk_r", [1, DSL], BF16, kind="ExternalInput")
    bv_r = nc.dram_tensor("bv_r", [1, DSL], BF16, kind="ExternalInput")
    srow = nc.dram_tensor("srow", [1, N], BF16, kind="ExternalInput")
    onesrow = nc.dram_tensor("onesrow", [1, 512], BF16, kind="ExternalInput")
    ones64 = nc.dram_tensor("ones64", [65, 64], F32R, kind="ExternalInput")
    onescol = nc.dram_tensor("onescol", [128, 1], BF16, kind="ExternalInput")
    identb = nc.dram_tensor("identb", [128, 128], BF16, kind="ExternalInput")
    out_part = nc.dram_tensor("out_part", [N, D], F32, kind="ExternalOutput")

    with tile.TileContext(nc) as tc:
        with tc.tile_pool(name="consts", bufs=1) as consts, \
             tc.tile_pool(name="persist", bufs=1) as persist, \
             tc.tile_pool(name="stream", bufs=4) as stream, \
             tc.tile_pool(name="work", bufs=3) as work, \
             tc.tile_pool(name="outp", bufs=2) as outp, \
             tc.tile_pool(name="psum", bufs=1, space="PSUM") as pp:

            # ---- constants -------------------------------------------------
            xT_sb = consts.tile([128, 8, N], BF16, tag="xT")
            xkT_sb = consts.tile([128, 8, N], BF16, tag="xkT")
            xk_r = xkT.rearrange("(t p) n -> p t n", p=128)
            x_r = xT.rearrange("(t p) n -> p t n", p=128)
            for c in range(4):
                nsl = slice(c * 512, c * 512 + 512)
                nc.sync.dma_start(out=xkT_sb[:, :, nsl], in_=xk_r[:, :, nsl])
            for c in range(4):
                nsl = slice(c * 512, c * 512 + 512)
                nc.sync.dma_start(out=xT_sb[:, :, nsl], in_=x_r[:, :, nsl])
            wq_t = consts.tile([128, 8, DSL], BF16, tag="wq")
            wk_t = consts.tile([128, 8, DSL], BF16, tag="wk")
            wv_t = consts.tile([128, 8, DSL], BF16, tag="wv")
            nc.sync.dma_start(out=wq_t, in_=wq.rearrange("(t p) j -> p t j", p=128))
            nc.sync.dma_start(out=wk_t, in_=wk.rearrange("(t p) j -> p t j", p=128))
            nc.sync.dma_start(out=wv_t, in_=wv.rearrange("(t p) j -> p t j", p=128))
            wo_t = consts.tile([128, 2, D], BF16, tag="wo")
            nc.sync.dma_start(out=wo_t, in_=wo.rearrange("(t p) j -> p t j", p=128))
            identb_t = consts.tile([128, 128], BF16, tag="identb")
            nc.sync.dma_start(out=identb_t, in_=identb[:])
            bq_t = consts.tile([1, DSL], BF16, tag="bq")
            bk_t = consts.tile([1, DSL], BF16, tag="bk")
            bv_t = consts.tile([1, DSL], BF16, tag="bv")
            srow_t = consts.tile([1, N], BF16, tag="srow")
            ones_t = consts.tile([1, 512], BF16, tag="ones")
            ones64_t = consts.tile([65, 64], F32R, tag="ones64")
            onescol_t = consts.tile([128, 1], BF16, tag="onescol")
            nc.sync.dma_start(out=bq_t, in_=bq_r[:])
            nc.sync.dma_start(out=bk_t, in_=bk_r[:])
            nc.sync.dma_start(out=bv_t, in_=bv_r[:])
            nc.sync.dma_start(out=srow_t, in_=srow[:])
            nc.sync.dma_start(out=ones_t, in_=onesrow[:])
            nc.sync.dma_start(out=ones64_t, in_=ones64[:])
            nc.sync.dma_start(out=onescol_t, in_=onescol[:])

            # ---- persistent intermediates ---------------------------------
            # q^T/k^T: [dk-pair row (h%2)*64+dk, hp, n]
            qT_all = persist.tile([128, 2, N], BF16, tag="qT")
            kT_all = persist.tile([128, 2, N], BF16, tag="kT")
            # v natural + ones col: [m-part, m-tile, head, 65]
            vaug = persist.tile([128, MT, HPC, 65], BF16, tag="vaug")
            # normalized y^T for O-proj
            yT_all = persist.tile([128, 2, N], BF16, tag="yT")
            # per-quarter y + denominators staging
            y_sb = persist.tile([65, HPC, 512], F32R, tag="ysb")

            # vaug ones columns, written once
            nc.vector.tensor_copy(
                vaug[:, :, :, 64:65],
                onescol_t.unsqueeze(1).unsqueeze(1).broadcast_to([128, MT, HPC, 1]))

            # ---- phase 1: projections (K all, V all, Q all) ---------------
            # K chunks
            for c in range(4):
                nsl = slice(c * 512, c * 512 + 512)
                ps = pp.tile([128, 2, 512], F32, tag="s", bufs=2, name=f"kps{c}")
                for hp in range(2):
                    csl = slice(hp * 128, hp * 128 + 128)
                    for t in range(8):
                        nc.tensor.matmul(
                            ps[:, hp], wk_t[:, t, csl], xkT_sb[:, t, nsl],
                            start=(t == 0), stop=False)
                    nc.tensor.matmul(
                        ps[:, hp], bk_t[0:1, csl], srow_t[0:1, nsl],
                        start=False, stop=True)
                nc.vector.tensor_copy(kT_all[:, :, nsl], ps)

            # V m-tiles (PSUM staged in the y-tag slot, 4 subtiles ping-pong)
            vps = pp.tile([128, 4, 256], F32, tag="y", bufs=1, name="vps")
            for mt in range(MT):
                msl = slice(mt * 128, mt * 128 + 128)
                vp = vps[:, mt % 4, :]
                for t in range(8):
                    nc.tensor.matmul(
                        vp, xkT_sb[:, t, msl], wv_t[:, t, :],
                        start=(t == 0), stop=False)
                nc.tensor.matmul(
                    vp, srow_t[0:1, msl], bv_t[0:1, :], start=False, stop=True)
                vr = vp.rearrange("p (h d) -> p h d", h=HPC)
                if mt % 2 == 0:
                    nc.vector.tensor_copy(vaug[:, mt, :, 0:64], vr)
                else:
                    nc.scalar.activation(vaug[:, mt, :, 0:64], vr, Copy)

            # Q chunks (scale 1/sqrt(DK)=0.125 folded into the copy)
            for c in range(4):
                nsl = slice(c * 512, c * 512 + 512)
                ps = pp.tile([128, 2, 512], F32, tag="s", bufs=2, name=f"qps{c}")
                for hp in range(2):
                    csl = slice(hp * 128, hp * 128 + 128)
                    for t in range(8):
                        nc.tensor.matmul(
                            ps[:, hp], wq_t[:, t, csl], xT_sb[:, t, nsl],
                            start=(t == 0), stop=False)
                    nc.tensor.matmul(
                        ps[:, hp], bq_t[0:1, csl], ones_t[0:1, :],
                        start=False, stop=True)
                nc.vector.tensor_scalar_mul(qT_all[:, :, nsl], ps, 0.125)

            # ---- phase 2+3, software-pipelined across quarters ------------
            n_rounds = MT * 2
            state = {}

            def qk_round(q4, r):
                nsl = slice(q4 * 512, q4 * 512 + 512)
                mt, rr = divmod(r, 2)
                pe_bias = PE_BIAS_ROUND[r]
                if rr == 0:
                    b_t = stream.tile([128, 512], BF16, tag="bias",
                                      name=f"b{q4}_{mt}")
                    nc.sync.dma_start(
                        out=b_t, in_=biasT[mt * 128:mt * 128 + 128, nsl])
                    state["b_cur"] = b_t
                b_t = state["b_cur"]
                s_ps = pp.tile([128, 2, 512], F32, tag="s", bufs=2,
                               name=f"s{q4}_{r}")
                for hi in range(2):
                    h = rr * 2 + hi
                    hp = h // 2
                    rsl = slice((h % 2) * 64, (h % 2) * 64 + 64)
                    nc.tensor.matmul(
                        s_ps[:, hi],
                        kT_all[rsl, hp, mt * 128:mt * 128 + 128],
                        qT_all[rsl, hp, nsl],
                        start=True, stop=(not pe_bias))
                e_t = work.tile([128, 2, 512], BF16, tag="e", bufs=3,
                                name=f"e{q4}_{r}")
                if pe_bias:
                    for hi in range(2):
                        nc.tensor.matmul(
                            s_ps[:, hi], identb_t, b_t, start=False, stop=True)
                    nc.scalar.activation(e_t, s_ps, Exp)
                else:
                    sb_s = work.tile([128, 2, 512], F32, tag="sbs", bufs=2,
                                     name=f"sb{q4}_{r}")
                    nc.vector.tensor_tensor(
                        sb_s, s_ps,
                        b_t.unsqueeze(1).broadcast_to([128, 2, 512]), Add)
                    nc.scalar.activation(e_t, sb_s, Exp)
                state[("e", r % 3)] = e_t

            def av_round(q4, r):
                mt, rr = divmod(r, 2)
                e_t = state[("e", r % 3)]
                y_ps = state[("y", q4)]
                for hi in range(2):
                    h = rr * 2 + hi
                    nc.tensor.matmul(
                        y_ps[:, h], vaug[:, mt, h, :], e_t[:, hi],
                        start=(mt == 0), stop=(mt == MT - 1))

            def tail_a(q4):
                # y -> SBUF, then reshape denom row onto 128 partitions via
                # SBUF->SBUF DMA, tiny DVE reciprocal, DMA back to a row.
                y_ps = state.pop(("y", q4))
                nc.vector.tensor_copy(y_sb, y_ps)
                d_t = work.tile([128, 16], F32R, tag="dt", bufs=1,
                                name=f"dt{q4}")
                nc.sync.dma_start(out=d_t, in_=y_sb[64:65, :, :])
                d_r = work.tile([128, 16], F32R, tag="dr", bufs=1,
                                name=f"dr{q4}")
                nc.vector.reciprocal(out=d_r.bitcast(F32), in_=d_t.bitcast(F32))
                r_row = work.tile([1, HPC, 512], F32R, tag="rrow", bufs=1,
                                  name=f"rr{q4}")
                nc.sync.dma_start(out=r_row, in_=d_r)
                state[("rrow", q4)] = r_row

            def tail_b(q4):
                r_row = state.pop(("rrow", q4))
                for hq in range(2):
                    rb = pp.tile([128, 2, 512], F32, tag="s", bufs=2,
                                 name=f"rb{q4}_{hq}")
                    for hi in range(2):
                        h = hq * 2 + hi
                        nc.tensor.matmul(
                            rb[0:64, hi, :], ones64_t[0:1, :],
                            r_row[0:1, h, :], start=True, stop=True)
                    for hi in range(2):
                        h = hq * 2 + hi
                        hp, hi2 = divmod(h, 2)
                        nc.vector.tensor_mul(
                            yT_all[hi2 * 64:hi2 * 64 + 64, hp,
                                   q4 * 512:q4 * 512 + 512],
                            y_sb[0:64, h, :].bitcast(F32), rb[0:64, hi, :])

            def oproj(q4, j):
                nt = q4 * 4 + j
                o_ps = pp.tile([128, 2, 512], F32, tag="s", bufs=2,
                               name=f"o{nt}")
                for dc in range(2):
                    for hp in range(2):
                        nc.tensor.matmul(
                            o_ps[:, dc],
                            yT_all[:, hp, nt * 128:nt * 128 + 128],
                            wo_t[:, hp, dc * 512:dc * 512 + 512],
                            start=(hp == 0), stop=(hp == 1))
                o_sb = outp.tile([128, D], F32, tag="osb", name=f"ob{nt}")
                nc.vector.tensor_copy(o_sb.rearrange("p (c f) -> p c f", c=2),
                                      o_ps)
                nc.sync.dma_start(
                    out=out_part[nt * 128:nt * 128 + 128, :], in_=o_sb)

            for q4 in range(NQ4):
                state[("y", q4)] = pp.tile(
                    [65, HPC, 512], F32, tag="y", bufs=1, name=f"y_ps{q4}")
                qk_round(q4, 0)
                for r in range(1, n_rounds):
                    qk_round(q4, r)
                    av_round(q4, r - 1)
                    if q4 > 0:
                        # previous quarter's tail + O-proj, spread so PE
                        # never waits on the recip DMA chain
                        if r == 2:
                            tail_a(q4 - 1)
                        elif r == 8:
                            tail_b(q4 - 1)
                        elif r in (12, 16, 20, 24):
                            oproj(q4 - 1, (r - 12) // 4)
                av_round(q4, n_rounds - 1)
            tail_a(NQ4 - 1)
            tail_b(NQ4 - 1)
            for j in range(4):
                oproj(NQ4 - 1, j)

    return nc


def _ensure_ntff_hook():
    """Register the axon NTFF profiling hook if the agent image lacks
    antenv.axon_hooks (profiling only; kernel runs fine without)."""
    try:
        from antenv.axon_hooks import get_axon_ntff_profile_hook  # noqa: F401
        return
    except ImportError:
        pass
    import types
    import antenv
    from trn_agent_boot.trn_boot import _ntff_profile_via_ctypes

    mod = types.ModuleType("antenv.axon_hooks")
    holder = {}
    mod.set_axon_ntff_profile_hook = lambda h: holder.__setitem__("h", h)
    mod.get_axon_ntff_profile_hook = lambda: holder.get("h")
    sys.modules["antenv.axon_hooks"] = mod
    antenv.axon_hooks = mod
    mod.set_axon_ntff_profile_hook(
        _ntff_profile_via_ctypes("/opt/axon/libaxon_pjrt.so"))


_NC_CACHE: dict = {}


def _get_nc() -> bass.Bass:
    if "nc" not in _NC_CACHE:
        _NC_CACHE["nc"] = _build_nc()
    return _NC_CACHE["nc"]


def kernel(x, alpha, bias, Wq, bq, Wk, bk, Wv, bv, Wo, bo, trace=False):
    bf = ml_dtypes.bfloat16
    x = np.asarray(x, np.float32)
    alpha = np.asarray(alpha, np.float32)
    bias = np.asarray(bias, np.float32)
    Wq = np.asarray(Wq, np.float32); bq = np.asarray(bq, np.float32)
    Wk = np.asarray(Wk, np.float32); bk = np.asarray(bk, np.float32)
    Wv = np.asarray(Wv, np.float32); bv = np.asarray(bv, np.float32)
    Wo = np.asarray(Wo, np.float32); bo = np.asarray(bo, np.float32)

    c = np.ascontiguousarray

    in_maps = []
    per_b = {}
    for b in range(B):
        s = 1.0 + alpha[b]                             # (N,)
        per_b[b] = {
            "xT": c(x[b].T.astype(bf)),                # (D, N)
            "xkT": c((x[b] * s[:, None]).T.astype(bf)),
            "biasT": c(bias[b].T.astype(bf)),          # (N, N) [m, n]
            "srow": s.reshape(1, N).astype(bf),
        }
    for core in range(NCORES):
        b, hg = divmod(core, 4)
        dsl = slice(hg * DSL, hg * DSL + DSL)
        in_maps.append({
            **per_b[b],
            "wq": c(Wq[:, dsl].astype(bf)),
            "wk": c(Wk[:, dsl].astype(bf)),
            "wv": c(Wv[:, dsl].astype(bf)),
            "wo": c(Wo[dsl, :].astype(bf)),
            "bq_r": c(bq[dsl].reshape(1, DSL).astype(bf)),
            "bk_r": c(bk[dsl].reshape(1, DSL).astype(bf)),
            "bv_r": c(bv[dsl].reshape(1, DSL).astype(bf)),
            "onesrow": np.ones((1, 512), bf),
            "ones64": np.ones((65, 64), np.float32),
            "onescol": np.ones((128, 1), bf),
            "identb": np.eye(128, dtype=bf),
        })

    if trace:
        _ensure_ntff_hook()
    nc = _get_nc()
    res = run_bass_kernel_spmd(
        nc, in_maps, core_ids=list(range(NCORES)), trace=trace)

    out = np.zeros((B, N, D), np.float32)
    for core in range(NCORES):
        out[core // 4] += res.results[core]["out_part"]
    out += bo[None, None, :]
    if trace:
        kernel.last_exec_time_ns = res.exec_time_ns
        kernel.last_profile = res.profile_json
    return out


# revision 7
# speedup vs baseline: 1.5616x; 1.0922x over previous
"""Biased self-attention TRN2 Bass kernel (8 NeuronCores), v2.

Problem: nn_BiasedSelfAttention — B=2, N=2048, D=1024, H=16, DK=64.
    q,k,v = split_heads(x@Wq+bq), ...; k,v scaled by (1+alpha[b,n]);
    logits = q k^T/sqrt(DK) + bias[b][None]; y = softmax(logits) v;
    out = merge_heads(y) @ Wo + bo.

Sharding: 8 cores = (batch b in {0,1}) x (head-group hg in {0..3} of 4
heads = 256 dims of D).  Data parallel over B, tensor parallel over H.
Each core computes a partial O-projection (its 256 rows of Wo); the
host sums the 4 partials per batch and adds bo.

v2 design (from NTFF profile of v1: PE 88% busy incl. 4.3 GF of bias
identity-injects, ACT 53%, DVE 3%, 33% HAM-throttled):
  - all matmul operands bf16 (same PE rate as f32r, half the DMA/SBUF)
  - bias add alternates per-round between PE identity-inject (into
    PSUM) and DVE tensor_tensor add (PSUM+SBUF->SBUF), balancing the
    three engines; exp is one ACT instr per round over the head-pair
  - projection phase: K then V then Q, x/xk fully resident in SBUF
  - normalize tail: ones-row K=1 matmul broadcasts denominators,
    DVE reciprocal_approx_fast + mul; bo added on host
"""

import json
import sys

sys.path.insert(0, "/opt/trn_rl_repo")

import numpy as np
import ml_dtypes

import concourse.bass as bass
import concourse.mybir as mybir
import concourse.tile as tile
from concourse.bass_utils import run_bass_kernel_spmd

# ---------------------------------------------------------------- bir fix --
# The pinned walrus encodes at most ONE sem-wait per instruction, but Tile's
# wait-assigner can emit several.  Hoist extras onto EventSemaphore
# instructions just before the instruction.


def _split_multi_waits(bir_json: bytes) -> bytes:
    m = json.loads(bir_json)
    for fn in m.get("functions", []):
        for blk in fn.get("blocks", []):
            insts = blk.get("instructions")
            if not insts:
                continue
            out = []
            for inst in insts:
                sync = inst.get("sync_info")
                waits = (sync or {}).get("on_wait") or []
                if len(waits) > 1:
                    for i, w in enumerate(waits[:-1]):
                        out.append({
                            "debug": inst.get("debug", 0),
                            "engine": inst["engine"],
                            "ins": [],
                            "name": f"{inst['name']}-sw{i}",
                            "opcode": "EventSemaphore",
                            "outs": [],
                            "sync_info": {"on_update": [], "on_wait": [w]},
                        })
                    sync["on_wait"] = waits[-1:]
                out.append(inst)
            blk["instructions"] = out
    return json.dumps(m).encode()


def _patch_bass():
    if getattr(bass.Bass, "_multiwait_patched", False):
        return
    orig = bass.Bass.to_json_bytes

    def to_json_bytes(self, *a, **kw):
        return _split_multi_waits(orig(self, *a, **kw))

    bass.Bass.to_json_bytes = to_json_bytes
    bass.Bass._multiwait_patched = True


_patch_bass()

# ------------------------------------------------------------- dimensions --
B, N, D, H = 2, 2048, 1024, 16
DK = D // H                      # 64
NCORES = 8
HPC = H // 4                     # 4 heads per core
DSL = HPC * DK                   # 256 D-columns per core
NQ4 = N // 512                   # 4 query quarters
MT = N // 128                    # 16 key tiles
F32 = mybir.dt.float32
F32R = mybir.dt.float32r
BF16 = mybir.dt.bfloat16
Exp = mybir.ActivationFunctionType.Exp
Copy = mybir.ActivationFunctionType.Copy
Add = mybir.AluOpType.add

# round flavor: True -> bias injected on PE; False -> bias added on DVE.
# Alternate to balance PE (~852ns/round) vs DVE (~658ns/round) vs ACT
# (~1038ns/round exp).
PE_BIAS_ROUND = [(r % 2 == 0) or (r >= 28) for r in range(MT * 2)]


def _build_nc() -> bass.Bass:
    nc = bass.Bass()

    xT = nc.dram_tensor("xT", [D, N], BF16, kind="ExternalInput")
    xkT = nc.dram_tensor("xkT", [D, N], BF16, kind="ExternalInput")
    wq = nc.dram_tensor("wq", [D, DSL], BF16, kind="ExternalInput")
    wk = nc.dram_tensor("wk", [D, DSL], BF16, kind="ExternalInput")
    wv = nc.dram_tensor("wv", [D, DSL], BF16, kind="ExternalInput")
    wo = nc.dram_tensor("wo", [DSL, D], BF16, kind="ExternalInput")
    biasT = nc.dram_tensor("biasT", [N, N], BF16, kind="ExternalInput")
    bq_r = nc.dram_tensor("bq_r", [1, DSL], BF16, kind="ExternalInput")
    bk_r = nc.dram_tensor("bk_r", [1, DSL], BF16, kind="ExternalInput")
    bv_r = nc.dram_tensor("bv_r", [1, DSL], BF16, kind="ExternalInput")
    srow = nc.dram_tensor("srow", [1, N], BF16, kind="ExternalInput")
    onesrow = nc.dram_tensor("onesrow", [1, 512], BF16, kind="ExternalInput")
    ones64 = nc.dram_tensor("ones64", [65, 64], F32R, kind="ExternalInput")
    onescol = nc.dram_tensor("onescol", [128, 1], BF16, kind="ExternalInput")
    identb = nc.dram_tensor("identb", [128, 128], BF16, kind="ExternalInput")
    out_part = nc.dram_tensor("out_part", [N, D], F32, kind="ExternalOutput")

    with tile.TileContext(nc) as tc:
        with tc.tile_pool(name="consts", bufs=1) as consts, \
             tc.tile_pool(name="persist", bufs=1) as persist, \
             tc.tile_pool(name="stream", bufs=4) as stream, \
             tc.tile_pool(name="work", bufs=3) as work, \
             tc.tile_pool(name="outp", bufs=2) as outp, \
             tc.tile_pool(name="psum", bufs=1, space="PSUM") as pp:

            # ---- constants -------------------------------------------------
            xT_sb = consts.tile([128, 8, N], BF16, tag="xT")
            xkT_sb = consts.tile([128, 8, N], BF16, tag="xkT")
            wq_t = consts.tile([128, 8, DSL], BF16, tag="wq")
            wk_t = consts.tile([128, 8, DSL], BF16, tag="wk")
            wv_t = consts.tile([128, 8, DSL], BF16, tag="wv")
            wo_t = consts.tile([128, 2, D], BF16, tag="wo")
            identb_t = consts.tile([128, 128], BF16, tag="identb")
            bq_t = consts.tile([1, DSL], BF16, tag="bq")
            bk_t = consts.tile([1, DSL], BF16, tag="bk")
            bv_t = consts.tile([1, DSL], BF16, tag="bv")
            srow_t = consts.tile([1, N], BF16, tag="srow")
            ones_t = consts.tile([1, 512], BF16, tag="ones")
            ones64_t = consts.tile([65, 64], F32R, tag="ones64")
            onescol_t = consts.tile([128, 1], BF16, tag="onescol")
            xk_r = xkT.rearrange("(t p) n -> p t n", p=128)
            x_r = xT.rearrange("(t p) n -> p t n", p=128)
            # K path first: wk + srow/bk, then xk blocks so K-proj can start
            nc.sync.dma_start(out=wk_t, in_=wk.rearrange("(t p) j -> p t j", p=128))
            nc.sync.dma_start(out=bk_t, in_=bk_r[:])
            nc.sync.dma_start(out=srow_t, in_=srow[:])
            nc.sync.dma_start(out=wv_t, in_=wv.rearrange("(t p) j -> p t j", p=128))
            nc.sync.dma_start(out=bv_t, in_=bv_r[:])
            for c in range(4):
                nsl = slice(c * 512, c * 512 + 512)
                nc.sync.dma_start(out=xkT_sb[:, :, nsl], in_=xk_r[:, :, nsl])
            nc.sync.dma_start(out=wq_t, in_=wq.rearrange("(t p) j -> p t j", p=128))
            nc.sync.dma_start(out=bq_t, in_=bq_r[:])
            nc.sync.dma_start(out=ones_t, in_=onesrow[:])
            nc.sync.dma_start(out=identb_t, in_=identb[:])
            nc.sync.dma_start(out=ones64_t, in_=ones64[:])
            nc.sync.dma_start(out=onescol_t, in_=onescol[:])
            for c in range(4):
                nsl = slice(c * 512, c * 512 + 512)
                nc.sync.dma_start(out=xT_sb[:, :, nsl], in_=x_r[:, :, nsl])
            nc.sync.dma_start(out=wo_t, in_=wo.rearrange("(t p) j -> p t j", p=128))

            # ---- persistent intermediates ---------------------------------
            # q^T/k^T: [dk-pair row (h%2)*64+dk, hp, n]
            qT_all = persist.tile([128, 2, N], BF16, tag="qT")
            kT_all = persist.tile([128, 2, N], BF16, tag="kT")
            # v natural + ones col: [m-part, m-tile, head, 65]
            vaug = persist.tile([128, MT, HPC, 65], BF16, tag="vaug")
            # normalized y^T for O-proj
            yT_all = persist.tile([128, 2, N], BF16, tag="yT")
            # per-quarter y + denominators staging
            y_sb = persist.tile([65, HPC, 512], F32R, tag="ysb")

            # vaug ones columns, written once
            nc.vector.tensor_copy(
                vaug[:, :, :, 64:65],
                onescol_t.unsqueeze(1).unsqueeze(1).broadcast_to([128, MT, HPC, 1]))

            # ---- phase 1: projections (K all, V all, Q all) ---------------
            # K chunks
            for c in range(4):
                nsl = slice(c * 512, c * 512 + 512)
                ps = pp.tile([128, 2, 512], F32, tag="s", bufs=2, name=f"kps{c}")
                for hp in range(2):
                    csl = slice(hp * 128, hp * 128 + 128)
                    for t in range(8):
                        nc.tensor.matmul(
                            ps[:, hp], wk_t[:, t, csl], xkT_sb[:, t, nsl],
                            start=(t == 0), stop=False)
                    nc.tensor.matmul(
                        ps[:, hp], bk_t[0:1, csl], srow_t[0:1, nsl],
                        start=False, stop=True)
                nc.vector.tensor_copy(kT_all[:, :, nsl], ps)

            # V m-tiles (PSUM staged in the y-tag slot, 4 subtiles ping-pong)
            vps = pp.tile([128, 4, 256], F32, tag="y", bufs=1, name="vps")
            for mt in range(MT):
                msl = slice(mt * 128, mt * 128 + 128)
                vp = vps[:, mt % 4, :]
                for t in range(8):
                    nc.tensor.matmul(
                        vp, xkT_sb[:, t, msl], wv_t[:, t, :],
                        start=(t == 0), stop=False)
                nc.tensor.matmul(
                    vp, srow_t[0:1, msl], bv_t[0:1, :], start=False, stop=True)
                vr = vp.rearrange("p (h d) -> p h d", h=HPC)
                if mt % 2 == 0:
                    nc.vector.tensor_copy(vaug[:, mt, :, 0:64], vr)
                else:
                    nc.scalar.activation(vaug[:, mt, :, 0:64], vr, Copy)

            # Q chunks (scale 1/sqrt(DK)=0.125 folded into the copy)
            for c in range(4):
                nsl = slice(c * 512, c * 512 + 512)
                ps = pp.tile([128, 2, 512], F32, tag="s", bufs=2, name=f"qps{c}")
                for hp in range(2):
                    csl = slice(hp * 128, hp * 128 + 128)
                    for t in range(8):
                        nc.tensor.matmul(
                            ps[:, hp], wq_t[:, t, csl], xT_sb[:, t, nsl],
                            start=(t == 0), stop=False)
                    nc.tensor.matmul(
                        ps[:, hp], bq_t[0:1, csl], ones_t[0:1, :],
                        start=False, stop=True)
                nc.vector.tensor_scalar_mul(qT_all[:, :, nsl], ps, 0.125)

            # ---- phase 2+3, software-pipelined across quarters ------------
            n_rounds = MT * 2
            state = {}

            def qk_round(q4, r):
                nsl = slice(q4 * 512, q4 * 512 + 512)
                mt, rr = divmod(r, 2)
                pe_bias = PE_BIAS_ROUND[r]
                if rr == 0:
                    b_t = stream.tile([128, 512], BF16, tag="bias",
                                      name=f"b{q4}_{mt}")
                    nc.sync.dma_start(
                        out=b_t, in_=biasT[mt * 128:mt * 128 + 128, nsl])
                    state["b_cur"] = b_t
                b_t = state["b_cur"]
                s_ps = pp.tile([128, 2, 512], F32, tag="s", bufs=2,
                               name=f"s{q4}_{r}")
                for hi in range(2):
                    h = rr * 2 + hi
                    hp = h // 2
                    rsl = slice((h % 2) * 64, (h % 2) * 64 + 64)
                    nc.tensor.matmul(
                        s_ps[:, hi],
                        kT_all[rsl, hp, mt * 128:mt * 128 + 128],
                        qT_all[rsl, hp, nsl],
                        start=True, stop=(not pe_bias))
                e_t = work.tile([128, 2, 512], BF16, tag="e", bufs=3,
                                name=f"e{q4}_{r}")
                if pe_bias:
                    for hi in range(2):
                        nc.tensor.matmul(
                            s_ps[:, hi], identb_t, b_t, start=False, stop=True)
                    nc.scalar.activation(e_t, s_ps, Exp)
                else:
                    sb_s = work.tile([128, 2, 512], F32, tag="sbs", bufs=4,
                                     name=f"sb{q4}_{r}")
                    nc.vector.tensor_tensor(
                        sb_s, s_ps,
                        b_t.unsqueeze(1).broadcast_to([128, 2, 512]), Add)
                    nc.scalar.activation(e_t, sb_s, Exp)
                state[("e", r % 3)] = e_t

            def av_round(q4, r):
                mt, rr = divmod(r, 2)
                e_t = state[("e", r % 3)]
                y_ps = state[("y", q4)]
                for hi in range(2):
                    h = rr * 2 + hi
                    nc.tensor.matmul(
                        y_ps[:, h], vaug[:, mt, h, :], e_t[:, hi],
                        start=(mt == 0), stop=(mt == MT - 1))

            def tail_a(q4):
                # y -> SBUF, then reshape denom row onto 128 partitions via
                # SBUF->SBUF DMA, tiny DVE reciprocal, DMA back to a row.
                y_ps = state.pop(("y", q4))
                nc.vector.tensor_copy(y_sb, y_ps)
                d_t = work.tile([128, 16], F32R, tag="dt", bufs=1,
                                name=f"dt{q4}")
                nc.sync.dma_start(out=d_t, in_=y_sb[64:65, :, :])
                d_r = work.tile([128, 16], F32R, tag="dr", bufs=1,
                                name=f"dr{q4}")
                nc.vector.reciprocal(out=d_r.bitcast(F32), in_=d_t.bitcast(F32))
                r_row = work.tile([1, HPC, 512], F32R, tag="rrow", bufs=1,
                                  name=f"rr{q4}")
                nc.sync.dma_start(out=r_row, in_=d_r)
                state[("rrow", q4)] = r_row

            def tail_b(q4, hq):
                # one head-pair: 2 broadcast matmuls (PE) + 2 muls (DVE)
                r_row = state[("rrow", q4)]
                rb = pp.tile([128, 2, 512], F32, tag="s", bufs=2,
                             name=f"rb{q4}_{hq}")
                for hi in range(2):
                    h = hq * 2 + hi
                    nc.tensor.matmul(
                        rb[0:64, hi, :], ones64_t[0:1, :],
                        r_row[0:1, h, :], start=True, stop=True)
                for hi in range(2):
                    h = hq * 2 + hi
                    hp, hi2 = divmod(h, 2)
                    nc.vector.tensor_mul(
                        yT_all[hi2 * 64:hi2 * 64 + 64, hp,
                               q4 * 512:q4 * 512 + 512],
                        y_sb[0:64, h, :].bitcast(F32), rb[0:64, hi, :])

            def oproj(q4, j):
                nt = q4 * 4 + j
                o_ps = pp.tile([128, 2, 512], F32, tag="s", bufs=2,
                               name=f"o{nt}")
                for dc in range(2):
                    for hp in range(2):
                        nc.tensor.matmul(
                            o_ps[:, dc],
                            yT_all[:, hp, nt * 128:nt * 128 + 128],
                            wo_t[:, hp, dc * 512:dc * 512 + 512],
                            start=(hp == 0), stop=(hp == 1))
                o_sb = outp.tile([128, D], F32, tag="osb", name=f"ob{nt}")
                nc.scalar.activation(o_sb.rearrange("p (c f) -> p c f", c=2),
                                     o_ps, Copy)
                nc.sync.dma_start(
                    out=out_part[nt * 128:nt * 128 + 128, :], in_=o_sb)

            for q4 in range(NQ4):
                state[("y", q4)] = pp.tile(
                    [65, HPC, 512], F32, tag="y", bufs=1, name=f"y_ps{q4}")
                qk_round(q4, 0)
                for r in range(1, n_rounds):
                    qk_round(q4, r)
                    av_round(q4, r - 1)
                    if q4 > 0:
                        # previous quarter's tail + O-proj, spread so PE
                        # never waits on the recip DMA chain
                        if r == 2:
                            tail_a(q4 - 1)
                        elif r in (8, 10):
                            tail_b(q4 - 1, (r - 8) // 2)
                        elif r in (14, 18, 22, 26):
                            oproj(q4 - 1, (r - 14) // 4)
                av_round(q4, n_rounds - 1)
            tail_a(NQ4 - 1)
            tail_b(NQ4 - 1, 0)
            tail_b(NQ4 - 1, 1)
            for j in range(4):
                oproj(NQ4 - 1, j)

    return nc


def _ensure_ntff_hook():
    """Register the axon NTFF profiling hook if the agent image lacks
    antenv.axon_hooks (profiling only; kernel runs fine without)."""
    try:
        from antenv.axon_hooks import get_axon_ntff_profile_hook  # noqa: F401
        return
    except ImportError:
        pass
    import types
    import antenv
    from trn_agent_boot.trn_boot import _ntff_profile_via_ctypes

    mod = types.ModuleType("antenv.axon_hooks")
    holder = {}
    mod.set_axon_ntff_profile_hook = lambda h: holder.__setitem__("h", h)
    mod.get_axon_ntff_profile_hook = lambda: holder.get("h")
    sys.modules["antenv.axon_hooks"] = mod
    antenv.axon_hooks = mod
    mod.set_axon_ntff_profile_hook(
        _ntff_profile_via_ctypes("/opt/axon/libaxon_pjrt.so"))


_NC_CACHE: dict = {}


def _get_nc() -> bass.Bass:
    if "nc" not in _NC_CACHE:
        _NC_CACHE["nc"] = _build_nc()
    return _NC_CACHE["nc"]


def kernel(x, alpha, bias, Wq, bq, Wk, bk, Wv, bv, Wo, bo, trace=False):
    bf = ml_dtypes.bfloat16
    x = np.asarray(x, np.float32)
    alpha = np.asarray(alpha, np.float32)
    bias = np.asarray(bias, np.float32)
    Wq = np.asarray(Wq, np.float32); bq = np.asarray(bq, np.float32)
    Wk = np.asarray(Wk, np.float32); bk = np.asarray(bk, np.float32)
    Wv = np.asarray(Wv, np.float32); bv = np.asarray(bv, np.float32)
    Wo = np.asarray(Wo, np.float32); bo = np.asarray(bo, np.float32)

    c = np.ascontiguousarray

    in_maps = []
    per_b = {}
    for b in range(B):
        s = 1.0 + alpha[b]                             # (N,)
        per_b[b] = {
            "xT": c(x[b].T.astype(bf)),                # (D, N)
            "xkT": c((x[b] * s[:, None]).T.astype(bf)),
            "biasT": c(bias[b].T.astype(bf)),          # (N, N) [m, n]
            "srow": s.reshape(1, N).astype(bf),
        }
    for core in range(NCORES):
        b, hg = divmod(core, 4)
        dsl = slice(hg * DSL, hg * DSL + DSL)
        in_maps.append({
            **per_b[b],
            "wq": c(Wq[:, dsl].astype(bf)),
            "wk": c(Wk[:, dsl].astype(bf)),
            "wv": c(Wv[:, dsl].astype(bf)),
            "wo": c(Wo[dsl, :].astype(bf)),
            "bq_r": c(bq[dsl].reshape(1, DSL).astype(bf)),
            "bk_r": c(bk[dsl].reshape(1, DSL).astype(bf)),
            "bv_r": c(bv[dsl].reshape(1, DSL).astype(bf)),
            "onesrow": np.ones((1, 512), bf),
            "ones64": np.ones((65, 64), np.float32),
            "onescol": np.ones((128, 1), bf),
            "identb": np.eye(128, dtype=bf),
        })

    if trace:
        _ensure_ntff_hook()
    nc = _get_nc()
    res = run_bass_kernel_spmd(
        nc, in_maps, core_ids=list(range(NCORES)), trace=trace)

    out = np.zeros((B, N, D), np.float32)
    for core in range(NCORES):
        out[core // 4] += res.results[core]["out_part"]
    out += bo[None, None, :]
    if trace:
        kernel.last_exec_time_ns = res.exec_time_ns
        kernel.last_profile = res.profile_json
    return out


# revision 8
# speedup vs baseline: 1.6754x; 1.0729x over previous
"""Biased self-attention TRN2 Bass kernel (8 NeuronCores), v2.

Problem: nn_BiasedSelfAttention — B=2, N=2048, D=1024, H=16, DK=64.
    q,k,v = split_heads(x@Wq+bq), ...; k,v scaled by (1+alpha[b,n]);
    logits = q k^T/sqrt(DK) + bias[b][None]; y = softmax(logits) v;
    out = merge_heads(y) @ Wo + bo.

Sharding: 8 cores = (batch b in {0,1}) x (head-group hg in {0..3} of 4
heads = 256 dims of D).  Data parallel over B, tensor parallel over H.
Each core computes a partial O-projection (its 256 rows of Wo); the
host sums the 4 partials per batch and adds bo.

v2 design (from NTFF profile of v1: PE 88% busy incl. 4.3 GF of bias
identity-injects, ACT 53%, DVE 3%, 33% HAM-throttled):
  - all matmul operands bf16 (same PE rate as f32r, half the DMA/SBUF)
  - bias add alternates per-round between PE identity-inject (into
    PSUM) and DVE tensor_tensor add (PSUM+SBUF->SBUF), balancing the
    three engines; exp is one ACT instr per round over the head-pair
  - projection phase: K then V then Q, x/xk fully resident in SBUF
  - normalize tail: ones-row K=1 matmul broadcasts denominators,
    DVE reciprocal_approx_fast + mul; bo added on host
"""

import json
import sys

sys.path.insert(0, "/opt/trn_rl_repo")

import numpy as np
import ml_dtypes

import concourse.bass as bass
import concourse.mybir as mybir
import concourse.tile as tile
from concourse.bass_utils import run_bass_kernel_spmd

# ---------------------------------------------------------------- bir fix --
# The pinned walrus encodes at most ONE sem-wait per instruction, but Tile's
# wait-assigner can emit several.  Hoist extras onto EventSemaphore
# instructions just before the instruction.


def _split_multi_waits(bir_json: bytes) -> bytes:
    m = json.loads(bir_json)
    for fn in m.get("functions", []):
        for blk in fn.get("blocks", []):
            insts = blk.get("instructions")
            if not insts:
                continue
            out = []
            for inst in insts:
                sync = inst.get("sync_info")
                waits = (sync or {}).get("on_wait") or []
                if len(waits) > 1:
                    for i, w in enumerate(waits[:-1]):
                        out.append({
                            "debug": inst.get("debug", 0),
                            "engine": inst["engine"],
                            "ins": [],
                            "name": f"{inst['name']}-sw{i}",
                            "opcode": "EventSemaphore",
                            "outs": [],
                            "sync_info": {"on_update": [], "on_wait": [w]},
                        })
                    sync["on_wait"] = waits[-1:]
                out.append(inst)
            blk["instructions"] = out
    return json.dumps(m).encode()


def _patch_bass():
    if getattr(bass.Bass, "_multiwait_patched", False):
        return
    orig = bass.Bass.to_json_bytes

    def to_json_bytes(self, *a, **kw):
        return _split_multi_waits(orig(self, *a, **kw))

    bass.Bass.to_json_bytes = to_json_bytes
    bass.Bass._multiwait_patched = True


_patch_bass()

# ------------------------------------------------------------- dimensions --
B, N, D, H = 2, 2048, 1024, 16
DK = D // H                      # 64
NCORES = 8
HPC = H // 4                     # 4 heads per core
DSL = HPC * DK                   # 256 D-columns per core
NQ4 = N // 512                   # 4 query quarters
MT = N // 128                    # 16 key tiles
F32 = mybir.dt.float32
F32R = mybir.dt.float32r
BF16 = mybir.dt.bfloat16
Exp = mybir.ActivationFunctionType.Exp
Copy = mybir.ActivationFunctionType.Copy
Add = mybir.AluOpType.add

# round flavor: True -> bias injected on PE; False -> bias added on DVE.
# Alternate to balance PE (~852ns/round) vs DVE (~658ns/round) vs ACT
# (~1038ns/round exp).
PE_BIAS_ROUND = [((r // 2) % 2 == 0) or (r >= 28) for r in range(MT * 2)]


def _build_nc() -> bass.Bass:
    nc = bass.Bass()

    xT = nc.dram_tensor("xT", [128, 8, N], BF16, kind="ExternalInput")
    xkT = nc.dram_tensor("xkT", [128, 8, N], BF16, kind="ExternalInput")
    wq = nc.dram_tensor("wq", [128, 8, DSL], BF16, kind="ExternalInput")
    wk = nc.dram_tensor("wk", [128, 8, DSL], BF16, kind="ExternalInput")
    wv = nc.dram_tensor("wv", [128, 8, DSL], BF16, kind="ExternalInput")
    wo = nc.dram_tensor("wo", [128, 2, D], BF16, kind="ExternalInput")
    biasT = nc.dram_tensor("biasT", [N, N], BF16, kind="ExternalInput")
    bq_r = nc.dram_tensor("bq_r", [1, DSL], BF16, kind="ExternalInput")
    bk_r = nc.dram_tensor("bk_r", [1, DSL], BF16, kind="ExternalInput")
    bv_r = nc.dram_tensor("bv_r", [1, DSL], BF16, kind="ExternalInput")
    srow = nc.dram_tensor("srow", [1, N], BF16, kind="ExternalInput")
    onesrow = nc.dram_tensor("onesrow", [1, 512], BF16, kind="ExternalInput")
    ones64 = nc.dram_tensor("ones64", [65, 64], F32R, kind="ExternalInput")
    onescol = nc.dram_tensor("onescol", [128, 1], BF16, kind="ExternalInput")
    identb = nc.dram_tensor("identb", [128, 128], BF16, kind="ExternalInput")
    out_part = nc.dram_tensor("out_part", [N, D], F32, kind="ExternalOutput")

    with tile.TileContext(nc) as tc:
        with tc.tile_pool(name="consts", bufs=1) as consts, \
             tc.tile_pool(name="persist", bufs=1) as persist, \
             tc.tile_pool(name="stream", bufs=4) as stream, \
             tc.tile_pool(name="work", bufs=3) as work, \
             tc.tile_pool(name="outp", bufs=2) as outp, \
             tc.tile_pool(name="psum", bufs=1, space="PSUM") as pp:

            # ---- constants -------------------------------------------------
            xT_sb = consts.tile([128, 8, N], BF16, tag="xT")
            xkT_sb = consts.tile([128, 8, N], BF16, tag="xkT")
            wq_t = consts.tile([128, 8, DSL], BF16, tag="wq")
            wk_t = consts.tile([128, 8, DSL], BF16, tag="wk")
            wv_t = consts.tile([128, 8, DSL], BF16, tag="wv")
            wo_t = consts.tile([128, 2, D], BF16, tag="wo")
            identb_t = consts.tile([128, 128], BF16, tag="identb")
            bq_t = consts.tile([1, DSL], BF16, tag="bq")
            bk_t = consts.tile([1, DSL], BF16, tag="bk")
            bv_t = consts.tile([1, DSL], BF16, tag="bv")
            srow_t = consts.tile([1, N], BF16, tag="srow")
            ones_t = consts.tile([1, 512], BF16, tag="ones")
            ones64_t = consts.tile([65, 64], F32R, tag="ones64")
            onescol_t = consts.tile([128, 1], BF16, tag="onescol")
            xk_r = xkT
            x_r = xT
            # K path first: wk + srow/bk, then xk blocks so K-proj can start
            nc.sync.dma_start(out=wk_t, in_=wk[:])
            nc.sync.dma_start(out=bk_t, in_=bk_r[:])
            nc.sync.dma_start(out=srow_t, in_=srow[:])
            nc.sync.dma_start(out=wv_t, in_=wv[:])
            nc.sync.dma_start(out=bv_t, in_=bv_r[:])
            for c in range(4):
                nsl = slice(c * 512, c * 512 + 512)
                nc.sync.dma_start(out=xkT_sb[:, :, nsl], in_=xk_r[:, :, nsl])
            nc.sync.dma_start(out=wq_t, in_=wq[:])
            nc.sync.dma_start(out=bq_t, in_=bq_r[:])
            nc.sync.dma_start(out=ones_t, in_=onesrow[:])
            nc.sync.dma_start(out=identb_t, in_=identb[:])
            nc.sync.dma_start(out=ones64_t, in_=ones64[:])
            nc.sync.dma_start(out=onescol_t, in_=onescol[:])
            for c in range(4):
                nsl = slice(c * 512, c * 512 + 512)
                nc.sync.dma_start(out=xT_sb[:, :, nsl], in_=x_r[:, :, nsl])
            nc.sync.dma_start(out=wo_t, in_=wo[:])

            # ---- persistent intermediates ---------------------------------
            # q^T/k^T: [dk-pair row (h%2)*64+dk, hp, n]
            qT_all = persist.tile([128, 2, N], BF16, tag="qT")
            kT_all = persist.tile([128, 2, N], BF16, tag="kT")
            # v natural + ones col: [m-part, m-tile, head, 65]
            vaug = persist.tile([128, MT, HPC, 65], BF16, tag="vaug")
            # normalized y^T for O-proj
            yT_all = persist.tile([128, 2, N], BF16, tag="yT")
            # per-quarter y + denominators staging
            y_sb = persist.tile([65, HPC, 512], F32R, tag="ysb")

            # vaug ones columns, written once
            nc.vector.tensor_copy(
                vaug[:, :, :, 64:65],
                onescol_t.unsqueeze(1).unsqueeze(1).broadcast_to([128, MT, HPC, 1]))

            # ---- phase 1: projections, K/V interleaved per x-block --------
            vps = pp.tile([128, 4, 256], F32, tag="y", bufs=1, name="vps")

            def kproj(c):
                nsl = slice(c * 512, c * 512 + 512)
                ps = pp.tile([128, 2, 512], F32, tag="s", bufs=2, name=f"kps{c}")
                for hp in range(2):
                    csl = slice(hp * 128, hp * 128 + 128)
                    for t in range(8):
                        nc.tensor.matmul(
                            ps[:, hp], wk_t[:, t, csl], xkT_sb[:, t, nsl],
                            start=(t == 0), stop=False)
                    nc.tensor.matmul(
                        ps[:, hp], bk_t[0:1, csl], srow_t[0:1, nsl],
                        start=False, stop=True)
                nc.vector.tensor_copy(kT_all[:, :, nsl], ps)

            def vproj(mt):
                msl = slice(mt * 128, mt * 128 + 128)
                vp = vps[:, mt % 4, :]
                for t in range(8):
                    nc.tensor.matmul(
                        vp, xkT_sb[:, t, msl], wv_t[:, t, :],
                        start=(t == 0), stop=False)
                nc.tensor.matmul(
                    vp, srow_t[0:1, msl], bv_t[0:1, :], start=False, stop=True)
                vr = vp.rearrange("p (h d) -> p h d", h=HPC)
                if mt % 2 == 0:
                    nc.vector.tensor_copy(vaug[:, mt, :, 0:64], vr)
                else:
                    nc.scalar.activation(vaug[:, mt, :, 0:64], vr, Copy)

            for c in range(4):
                kproj(c)
                for mt in range(4 * c, 4 * c + 4):
                    vproj(mt)

            # Q chunks (scale 1/sqrt(DK)=0.125 folded into the copy)
            for c in range(4):
                nsl = slice(c * 512, c * 512 + 512)
                ps = pp.tile([128, 2, 512], F32, tag="s", bufs=2, name=f"qps{c}")
                for hp in range(2):
                    csl = slice(hp * 128, hp * 128 + 128)
                    for t in range(8):
                        nc.tensor.matmul(
                            ps[:, hp], wq_t[:, t, csl], xT_sb[:, t, nsl],
                            start=(t == 0), stop=False)
                    nc.tensor.matmul(
                        ps[:, hp], bq_t[0:1, csl], ones_t[0:1, :],
                        start=False, stop=True)
                nc.vector.tensor_scalar_mul(qT_all[:, :, nsl], ps, 0.125)

            # ---- phase 2+3, software-pipelined across quarters ------------
            n_rounds = MT * 2
            state = {}

            def qk_round(q4, r):
                nsl = slice(q4 * 512, q4 * 512 + 512)
                mt, rr = divmod(r, 2)
                pe_bias = PE_BIAS_ROUND[r]
                if rr == 0:
                    b_t = stream.tile([128, 512], BF16, tag="bias", bufs=6,
                                      name=f"b{q4}_{mt}")
                    nc.sync.dma_start(
                        out=b_t, in_=biasT[mt * 128:mt * 128 + 128, nsl])
                    state["b_cur"] = b_t
                b_t = state["b_cur"]
                s_ps = pp.tile([128, 2, 512], F32, tag="s", bufs=2,
                               name=f"s{q4}_{r}")
                for hi in range(2):
                    h = rr * 2 + hi
                    hp = h // 2
                    rsl = slice((h % 2) * 64, (h % 2) * 64 + 64)
                    nc.tensor.matmul(
                        s_ps[:, hi],
                        kT_all[rsl, hp, mt * 128:mt * 128 + 128],
                        qT_all[rsl, hp, nsl],
                        start=True, stop=(not pe_bias))
                e_t = work.tile([128, 2, 512], BF16, tag="e", bufs=4,
                                name=f"e{q4}_{r}")
                if pe_bias:
                    for hi in range(2):
                        nc.tensor.matmul(
                            s_ps[:, hi], identb_t, b_t, start=False, stop=True)
                    nc.scalar.activation(e_t, s_ps, Exp)
                else:
                    sb_s = work.tile([128, 2, 512], F32, tag="sbs", bufs=4,
                                     name=f"sb{q4}_{r}")
                    nc.vector.tensor_tensor(
                        sb_s, s_ps,
                        b_t.unsqueeze(1).broadcast_to([128, 2, 512]), Add)
                    nc.scalar.activation(e_t, sb_s, Exp)
                state[("e", r % 4)] = e_t

            def av_round(q4, r):
                mt, rr = divmod(r, 2)
                e_t = state[("e", r % 4)]
                y_ps = state[("y", q4)]
                for hi in range(2):
                    h = rr * 2 + hi
                    nc.tensor.matmul(
                        y_ps[:, h], vaug[:, mt, h, :], e_t[:, hi],
                        start=(mt == 0), stop=(mt == MT - 1))

            def tail_a(q4):
                # y -> SBUF, then reshape denom row onto 128 partitions via
                # SBUF->SBUF DMA, tiny DVE reciprocal, DMA back to a row.
                y_ps = state.pop(("y", q4))
                nc.vector.tensor_copy(y_sb, y_ps)
                d_t = work.tile([128, 16], F32R, tag="dt", bufs=1,
                                name=f"dt{q4}")
                nc.sync.dma_start(out=d_t, in_=y_sb[64:65, :, :])
                d_r = work.tile([128, 16], F32R, tag="dr", bufs=1,
                                name=f"dr{q4}")
                nc.vector.reciprocal(out=d_r.bitcast(F32), in_=d_t.bitcast(F32))
                r_row = work.tile([1, HPC, 512], F32R, tag="rrow", bufs=1,
                                  name=f"rr{q4}")
                nc.sync.dma_start(out=r_row, in_=d_r)
                state[("rrow", q4)] = r_row

            def tail_b(q4, hq):
                # one head-pair: 2 broadcast matmuls (PE) + 2 muls (DVE)
                r_row = state[("rrow", q4)]
                rb = pp.tile([128, 2, 512], F32, tag="s", bufs=2,
                             name=f"rb{q4}_{hq}")
                for hi in range(2):
                    h = hq * 2 + hi
                    nc.tensor.matmul(
                        rb[0:64, hi, :], ones64_t[0:1, :],
                        r_row[0:1, h, :], start=True, stop=True)
                for hi in range(2):
                    h = hq * 2 + hi
                    hp, hi2 = divmod(h, 2)
                    nc.vector.tensor_mul(
                        yT_all[hi2 * 64:hi2 * 64 + 64, hp,
                               q4 * 512:q4 * 512 + 512],
                        y_sb[0:64, h, :].bitcast(F32), rb[0:64, hi, :])

            def oproj(q4, j):
                nt = q4 * 4 + j
                o_ps = pp.tile([128, 2, 512], F32, tag="s", bufs=2,
                               name=f"o{nt}")
                for dc in range(2):
                    for hp in range(2):
                        nc.tensor.matmul(
                            o_ps[:, dc],
                            yT_all[:, hp, nt * 128:nt * 128 + 128],
                            wo_t[:, hp, dc * 512:dc * 512 + 512],
                            start=(hp == 0), stop=(hp == 1))
                o_sb = outp.tile([128, D], F32, tag="osb", name=f"ob{nt}")
                nc.scalar.activation(o_sb.rearrange("p (c f) -> p c f", c=2),
                                     o_ps, Copy)
                nc.sync.dma_start(
                    out=out_part[nt * 128:nt * 128 + 128, :], in_=o_sb)

            for q4 in range(NQ4):
                state[("y", q4)] = pp.tile(
                    [65, HPC, 512], F32, tag="y", bufs=1, name=f"y_ps{q4}")
                qk_round(q4, 0)
                for r in range(1, n_rounds):
                    qk_round(q4, r)
                    av_round(q4, r - 1)
                    if q4 > 0:
                        # previous quarter's tail + O-proj, spread so PE
                        # never waits on the recip DMA chain
                        if r == 2:
                            tail_a(q4 - 1)
                        elif r in (8, 10):
                            tail_b(q4 - 1, (r - 8) // 2)
                        elif r in (14, 18, 22, 26):
                            oproj(q4 - 1, (r - 14) // 4)
                av_round(q4, n_rounds - 1)
            tail_a(NQ4 - 1)
            tail_b(NQ4 - 1, 0)
            tail_b(NQ4 - 1, 1)
            for j in range(4):
                oproj(NQ4 - 1, j)

    return nc


def _ensure_ntff_hook():
    """Register the axon NTFF profiling hook if the agent image lacks
    antenv.axon_hooks (profiling only; kernel runs fine without)."""
    try:
        from antenv.axon_hooks import get_axon_ntff_profile_hook  # noqa: F401
        return
    except ImportError:
        pass
    import types
    import antenv
    from trn_agent_boot.trn_boot import _ntff_profile_via_ctypes

    mod = types.ModuleType("antenv.axon_hooks")
    holder = {}
    mod.set_axon_ntff_profile_hook = lambda h: holder.__setitem__("h", h)
    mod.get_axon_ntff_profile_hook = lambda: holder.get("h")
    sys.modules["antenv.axon_hooks"] = mod
    antenv.axon_hooks = mod
    mod.set_axon_ntff_profile_hook(
        _ntff_profile_via_ctypes("/opt/axon/libaxon_pjrt.so"))


_NC_CACHE: dict = {}


def _get_nc() -> bass.Bass:
    if "nc" not in _NC_CACHE:
        _NC_CACHE["nc"] = _build_nc()
    return _NC_CACHE["nc"]


def kernel(x, alpha, bias, Wq, bq, Wk, bk, Wv, bv, Wo, bo, trace=False):
    bf = ml_dtypes.bfloat16
    x = np.asarray(x, np.float32)
    alpha = np.asarray(alpha, np.float32)
    bias = np.asarray(bias, np.float32)
    Wq = np.asarray(Wq, np.float32); bq = np.asarray(bq, np.float32)
    Wk = np.asarray(Wk, np.float32); bk = np.asarray(bk, np.float32)
    Wv = np.asarray(Wv, np.float32); bv = np.asarray(bv, np.float32)
    Wo = np.asarray(Wo, np.float32); bo = np.asarray(bo, np.float32)

    c = np.ascontiguousarray

    in_maps = []
    per_b = {}
    for b in range(B):
        s = 1.0 + alpha[b]                             # (N,)
        xt = x[b].T.astype(bf)                         # (D, N)
        xkt = (x[b] * s[:, None]).T.astype(bf)
        per_b[b] = {
            # pre-arranged to SBUF layout [p, t, n] for 32KB-contiguous DMA
            "xT": c(xt.reshape(8, 128, N).transpose(1, 0, 2)),
            "xkT": c(xkt.reshape(8, 128, N).transpose(1, 0, 2)),
            "biasT": c(bias[b].T.astype(bf)),          # (N, N) [m, n]
            "srow": s.reshape(1, N).astype(bf),
        }
    for core in range(NCORES):
        b, hg = divmod(core, 4)
        dsl = slice(hg * DSL, hg * DSL + DSL)
        in_maps.append({
            **per_b[b],
            "wq": c(Wq[:, dsl].astype(bf).reshape(8, 128, DSL).transpose(1, 0, 2)),
            "wk": c(Wk[:, dsl].astype(bf).reshape(8, 128, DSL).transpose(1, 0, 2)),
            "wv": c(Wv[:, dsl].astype(bf).reshape(8, 128, DSL).transpose(1, 0, 2)),
            "wo": c(Wo[dsl, :].astype(bf).reshape(2, 128, D).transpose(1, 0, 2)),
            "bq_r": c(bq[dsl].reshape(1, DSL).astype(bf)),
            "bk_r": c(bk[dsl].reshape(1, DSL).astype(bf)),
            "bv_r": c(bv[dsl].reshape(1, DSL).astype(bf)),
            "onesrow": np.ones((1, 512), bf),
            "ones64": np.ones((65, 64), np.float32),
            "onescol": np.ones((128, 1), bf),
            "identb": np.eye(128, dtype=bf),
        })

    if trace:
        _ensure_ntff_hook()
    nc = _get_nc()
    res = run_bass_kernel_spmd(
        nc, in_maps, core_ids=list(range(NCORES)), trace=trace)

    out = np.zeros((B, N, D), np.float32)
    for core in range(NCORES):
        out[core // 4] += res.results[core]["out_part"]
    out += bo[None, None, :]
    if trace:
        kernel.last_exec_time_ns = res.exec_time_ns
        kernel.last_profile = res.profile_json
    return out


# revision 9
# speedup vs baseline: 1.6765x; 1.0007x over previous
"""Biased self-attention TRN2 Bass kernel (8 NeuronCores), v2.

Problem: nn_BiasedSelfAttention — B=2, N=2048, D=1024, H=16, DK=64.
    q,k,v = split_heads(x@Wq+bq), ...; k,v scaled by (1+alpha[b,n]);
    logits = q k^T/sqrt(DK) + bias[b][None]; y = softmax(logits) v;
    out = merge_heads(y) @ Wo + bo.

Sharding: 8 cores = (batch b in {0,1}) x (head-group hg in {0..3} of 4
heads = 256 dims of D).  Data parallel over B, tensor parallel over H.
Each core computes a partial O-projection (its 256 rows of Wo); the
host sums the 4 partials per batch and adds bo.

v2 design (from NTFF profile of v1: PE 88% busy incl. 4.3 GF of bias
identity-injects, ACT 53%, DVE 3%, 33% HAM-throttled):
  - all matmul operands bf16 (same PE rate as f32r, half the DMA/SBUF)
  - bias add alternates per-round between PE identity-inject (into
    PSUM) and DVE tensor_tensor add (PSUM+SBUF->SBUF), balancing the
    three engines; exp is one ACT instr per round over the head-pair
  - projection phase: K then V then Q, x/xk fully resident in SBUF
  - normalize tail: ones-row K=1 matmul broadcasts denominators,
    DVE reciprocal_approx_fast + mul; bo added on host
"""

import json
import sys

sys.path.insert(0, "/opt/trn_rl_repo")

import numpy as np
import ml_dtypes

import concourse.bass as bass
import concourse.mybir as mybir
import concourse.tile as tile
from concourse.bass_utils import run_bass_kernel_spmd

# ---------------------------------------------------------------- bir fix --
# The pinned walrus encodes at most ONE sem-wait per instruction, but Tile's
# wait-assigner can emit several.  Hoist extras onto EventSemaphore
# instructions just before the instruction.


def _split_multi_waits(bir_json: bytes) -> bytes:
    m = json.loads(bir_json)
    for fn in m.get("functions", []):
        for blk in fn.get("blocks", []):
            insts = blk.get("instructions")
            if not insts:
                continue
            out = []
            for inst in insts:
                sync = inst.get("sync_info")
                waits = (sync or {}).get("on_wait") or []
                if len(waits) > 1:
                    for i, w in enumerate(waits[:-1]):
                        out.append({
                            "debug": inst.get("debug", 0),
                            "engine": inst["engine"],
                            "ins": [],
                            "name": f"{inst['name']}-sw{i}",
                            "opcode": "EventSemaphore",
                            "outs": [],
                            "sync_info": {"on_update": [], "on_wait": [w]},
                        })
                    sync["on_wait"] = waits[-1:]
                out.append(inst)
            blk["instructions"] = out
    return json.dumps(m).encode()


def _patch_bass():
    if getattr(bass.Bass, "_multiwait_patched", False):
        return
    orig = bass.Bass.to_json_bytes

    def to_json_bytes(self, *a, **kw):
        return _split_multi_waits(orig(self, *a, **kw))

    bass.Bass.to_json_bytes = to_json_bytes
    bass.Bass._multiwait_patched = True


_patch_bass()

# ------------------------------------------------------------- dimensions --
B, N, D, H = 2, 2048, 1024, 16
DK = D // H                      # 64
NCORES = 8
HPC = H // 4                     # 4 heads per core
DSL = HPC * DK                   # 256 D-columns per core
NQ4 = N // 512                   # 4 query quarters
MT = N // 128                    # 16 key tiles
F32 = mybir.dt.float32
F32R = mybir.dt.float32r
BF16 = mybir.dt.bfloat16
Exp = mybir.ActivationFunctionType.Exp
Copy = mybir.ActivationFunctionType.Copy
Add = mybir.AluOpType.add

# round flavor: True -> bias injected on PE; False -> bias added on DVE.
# Alternate to balance PE (~852ns/round) vs DVE (~658ns/round) vs ACT
# (~1038ns/round exp).
PE_BIAS_ROUND = [((r // 2) % 2 == 0) or (r >= 28) for r in range(MT * 2)]


def _build_nc() -> bass.Bass:
    nc = bass.Bass()

    xT = nc.dram_tensor("xT", [128, 4, 8, 512], BF16, kind="ExternalInput")
    xkT = nc.dram_tensor("xkT", [128, 4, 8, 512], BF16, kind="ExternalInput")
    wq = nc.dram_tensor("wq", [128, 8, DSL], BF16, kind="ExternalInput")
    wk = nc.dram_tensor("wk", [128, 8, DSL], BF16, kind="ExternalInput")
    wv = nc.dram_tensor("wv", [128, 8, DSL], BF16, kind="ExternalInput")
    wo = nc.dram_tensor("wo", [128, 2, D], BF16, kind="ExternalInput")
    biasT = nc.dram_tensor("biasT", [N, N], BF16, kind="ExternalInput")
    bq_r = nc.dram_tensor("bq_r", [1, DSL], BF16, kind="ExternalInput")
    bk_r = nc.dram_tensor("bk_r", [1, DSL], BF16, kind="ExternalInput")
    bv_r = nc.dram_tensor("bv_r", [1, DSL], BF16, kind="ExternalInput")
    srow = nc.dram_tensor("srow", [1, N], BF16, kind="ExternalInput")
    onesrow = nc.dram_tensor("onesrow", [1, 512], BF16, kind="ExternalInput")
    ones64 = nc.dram_tensor("ones64", [65, 64], F32R, kind="ExternalInput")
    onescol = nc.dram_tensor("onescol", [128, 1], BF16, kind="ExternalInput")
    identb = nc.dram_tensor("identb", [128, 128], BF16, kind="ExternalInput")
    out_part = nc.dram_tensor("out_part", [N, D], F32, kind="ExternalOutput")

    with tile.TileContext(nc) as tc:
        with tc.tile_pool(name="consts", bufs=1) as consts, \
             tc.tile_pool(name="persist", bufs=1) as persist, \
             tc.tile_pool(name="stream", bufs=4) as stream, \
             tc.tile_pool(name="work", bufs=3) as work, \
             tc.tile_pool(name="outp", bufs=2) as outp, \
             tc.tile_pool(name="psum", bufs=1, space="PSUM") as pp:

            # ---- constants -------------------------------------------------
            xT_sb = consts.tile([128, 4, 8, 512], BF16, tag="xT")
            xkT_sb = consts.tile([128, 4, 8, 512], BF16, tag="xkT")
            wq_t = consts.tile([128, 8, DSL], BF16, tag="wq")
            wk_t = consts.tile([128, 8, DSL], BF16, tag="wk")
            wv_t = consts.tile([128, 8, DSL], BF16, tag="wv")
            wo_t = consts.tile([128, 2, D], BF16, tag="wo")
            identb_t = consts.tile([128, 128], BF16, tag="identb")
            bq_t = consts.tile([1, DSL], BF16, tag="bq")
            bk_t = consts.tile([1, DSL], BF16, tag="bk")
            bv_t = consts.tile([1, DSL], BF16, tag="bv")
            srow_t = consts.tile([1, N], BF16, tag="srow")
            ones_t = consts.tile([1, 512], BF16, tag="ones")
            ones64_t = consts.tile([65, 64], F32R, tag="ones64")
            onescol_t = consts.tile([128, 1], BF16, tag="onescol")
            # identity first (used to prewarm the PE while DMAs stream)
            nc.sync.dma_start(out=identb_t, in_=identb[:])
            # K path first: wk + srow/bk, then xk blocks so K-proj can start
            nc.sync.dma_start(out=wk_t, in_=wk[:])
            nc.sync.dma_start(out=bk_t, in_=bk_r[:])
            nc.sync.dma_start(out=srow_t, in_=srow[:])
            nc.sync.dma_start(out=wv_t, in_=wv[:])
            nc.sync.dma_start(out=bv_t, in_=bv_r[:])
            for c in range(4):
                nc.sync.dma_start(out=xkT_sb[:, c], in_=xkT[:, c])
            nc.sync.dma_start(out=wq_t, in_=wq[:])
            nc.sync.dma_start(out=bq_t, in_=bq_r[:])
            nc.sync.dma_start(out=ones_t, in_=onesrow[:])
            nc.sync.dma_start(out=ones64_t, in_=ones64[:])
            nc.sync.dma_start(out=onescol_t, in_=onescol[:])
            for c in range(4):
                nc.sync.dma_start(out=xT_sb[:, c], in_=xT[:, c])
            nc.sync.dma_start(out=wo_t, in_=wo[:])

            # ---- persistent intermediates ---------------------------------
            # q^T/k^T: [dk-pair row (h%2)*64+dk, hp, n]
            qT_all = persist.tile([128, 2, N], BF16, tag="qT")
            kT_all = persist.tile([128, 2, N], BF16, tag="kT")
            # v natural + ones col: [m-part, m-tile, head, 65]
            vaug = persist.tile([128, MT, HPC, 65], BF16, tag="vaug")
            # normalized y^T for O-proj
            yT_all = persist.tile([128, 2, N], BF16, tag="yT")
            # per-quarter y + denominators staging
            y_sb = persist.tile([65, HPC, 512], F32R, tag="ysb")

            # vaug ones columns, written once
            nc.vector.tensor_copy(
                vaug[:, :, :, 64:65],
                onescol_t.unsqueeze(1).unsqueeze(1).broadcast_to([128, MT, HPC, 1]))

            # ---- phase 1: projections, K/V interleaved per x-block --------
            vps = pp.tile([128, 4, 256], F32, tag="y", bufs=1, name="vps")

            def kproj(c):
                nsl = slice(c * 512, c * 512 + 512)
                ps = pp.tile([128, 2, 512], F32, tag="s", bufs=2, name=f"kps{c}")
                for hp in range(2):
                    csl = slice(hp * 128, hp * 128 + 128)
                    for t in range(8):
                        nc.tensor.matmul(
                            ps[:, hp], wk_t[:, t, csl], xkT_sb[:, c, t, :],
                            start=(t == 0), stop=False)
                    nc.tensor.matmul(
                        ps[:, hp], bk_t[0:1, csl], srow_t[0:1, nsl],
                        start=False, stop=True)
                nc.vector.tensor_copy(kT_all[:, :, nsl], ps)

            def vproj(mt):
                msl = slice(mt * 128, mt * 128 + 128)
                mb, mo = divmod(mt, 4)
                vp = vps[:, mt % 4, :]
                for t in range(8):
                    nc.tensor.matmul(
                        vp, xkT_sb[:, mb, t, mo * 128:mo * 128 + 128],
                        wv_t[:, t, :], start=(t == 0), stop=False)
                nc.tensor.matmul(
                    vp, srow_t[0:1, msl], bv_t[0:1, :], start=False, stop=True)
                vr = vp.rearrange("p (h d) -> p h d", h=HPC)
                if mt % 2 == 0:
                    nc.vector.tensor_copy(vaug[:, mt, :, 0:64], vr)
                else:
                    nc.scalar.activation(vaug[:, mt, :, 0:64], vr, Copy)

            warm = pp.tile([128, 2, 512], F32, tag="s", bufs=2, name="warm")
            for w in range(40):
                nc.tensor.matmul(warm[:, 0, 0:128], identb_t, identb_t,
                                 start=(w == 0), stop=(w == 39))

            for c in range(4):
                kproj(c)
                for mt in range(4 * c, 4 * c + 4):
                    vproj(mt)

            # Q chunks (scale 1/sqrt(DK)=0.125 folded into the copy)
            for c in range(4):
                nsl = slice(c * 512, c * 512 + 512)
                ps = pp.tile([128, 2, 512], F32, tag="s", bufs=2, name=f"qps{c}")
                for hp in range(2):
                    csl = slice(hp * 128, hp * 128 + 128)
                    for t in range(8):
                        nc.tensor.matmul(
                            ps[:, hp], wq_t[:, t, csl], xT_sb[:, c, t, :],
                            start=(t == 0), stop=False)
                    nc.tensor.matmul(
                        ps[:, hp], bq_t[0:1, csl], ones_t[0:1, :],
                        start=False, stop=True)
                nc.vector.tensor_scalar_mul(qT_all[:, :, nsl], ps, 0.125)

            # ---- phase 2+3, software-pipelined across quarters ------------
            n_rounds = MT * 2
            state = {}

            def qk_round(q4, r):
                nsl = slice(q4 * 512, q4 * 512 + 512)
                mt, rr = divmod(r, 2)
                pe_bias = PE_BIAS_ROUND[r]
                if rr == 0:
                    b_t = stream.tile([128, 512], BF16, tag="bias", bufs=6,
                                      name=f"b{q4}_{mt}")
                    nc.sync.dma_start(
                        out=b_t, in_=biasT[mt * 128:mt * 128 + 128, nsl])
                    state["b_cur"] = b_t
                b_t = state["b_cur"]
                s_ps = pp.tile([128, 2, 512], F32, tag="s", bufs=2,
                               name=f"s{q4}_{r}")
                for hi in range(2):
                    h = rr * 2 + hi
                    hp = h // 2
                    rsl = slice((h % 2) * 64, (h % 2) * 64 + 64)
                    nc.tensor.matmul(
                        s_ps[:, hi],
                        kT_all[rsl, hp, mt * 128:mt * 128 + 128],
                        qT_all[rsl, hp, nsl],
                        start=True, stop=(not pe_bias))
                e_t = work.tile([128, 2, 512], BF16, tag="e", bufs=4,
                                name=f"e{q4}_{r}")
                if pe_bias:
                    for hi in range(2):
                        nc.tensor.matmul(
                            s_ps[:, hi], identb_t, b_t, start=False, stop=True)
                    nc.scalar.activation(e_t, s_ps, Exp)
                else:
                    sb_s = work.tile([128, 2, 512], F32, tag="sbs", bufs=4,
                                     name=f"sb{q4}_{r}")
                    nc.vector.tensor_tensor(
                        sb_s, s_ps,
                        b_t.unsqueeze(1).broadcast_to([128, 2, 512]), Add)
                    nc.scalar.activation(e_t, sb_s, Exp)
                state[("e", r % 4)] = e_t

            def av_round(q4, r):
                mt, rr = divmod(r, 2)
                e_t = state[("e", r % 4)]
                y_ps = state[("y", q4)]
                for hi in range(2):
                    h = rr * 2 + hi
                    nc.tensor.matmul(
                        y_ps[:, h], vaug[:, mt, h, :], e_t[:, hi],
                        start=(mt == 0), stop=(mt == MT - 1))

            def tail_a(q4):
                # y -> SBUF, then reshape denom row onto 128 partitions via
                # SBUF->SBUF DMA, tiny DVE reciprocal, DMA back to a row.
                y_ps = state.pop(("y", q4))
                nc.vector.tensor_copy(y_sb, y_ps)
                d_t = work.tile([128, 16], F32R, tag="dt", bufs=1,
                                name=f"dt{q4}")
                nc.sync.dma_start(out=d_t, in_=y_sb[64:65, :, :])
                d_r = work.tile([128, 16], F32R, tag="dr", bufs=1,
                                name=f"dr{q4}")
                nc.vector.reciprocal(out=d_r.bitcast(F32), in_=d_t.bitcast(F32))
                r_row = work.tile([1, HPC, 512], F32R, tag="rrow", bufs=1,
                                  name=f"rr{q4}")
                nc.sync.dma_start(out=r_row, in_=d_r)
                state[("rrow", q4)] = r_row

            def tail_b(q4, hq):
                # one head-pair: 2 broadcast matmuls (PE) + 2 muls (DVE)
                r_row = state[("rrow", q4)]
                rb = pp.tile([128, 2, 512], F32, tag="s", bufs=2,
                             name=f"rb{q4}_{hq}")
                for hi in range(2):
                    h = hq * 2 + hi
                    nc.tensor.matmul(
                        rb[0:64, hi, :], ones64_t[0:1, :],
                        r_row[0:1, h, :], start=True, stop=True)
                for hi in range(2):
                    h = hq * 2 + hi
                    hp, hi2 = divmod(h, 2)
                    nc.vector.tensor_mul(
                        yT_all[hi2 * 64:hi2 * 64 + 64, hp,
                               q4 * 512:q4 * 512 + 512],
                        y_sb[0:64, h, :].bitcast(F32), rb[0:64, hi, :])

            def oproj(q4, j):
                nt = q4 * 4 + j
                o_ps = pp.tile([128, 2, 512], F32, tag="s", bufs=2,
                               name=f"o{nt}")
                for dc in range(2):
                    for hp in range(2):
                        nc.tensor.matmul(
                            o_ps[:, dc],
                            yT_all[:, hp, nt * 128:nt * 128 + 128],
                            wo_t[:, hp, dc * 512:dc * 512 + 512],
                            start=(hp == 0), stop=(hp == 1))
                o_sb = outp.tile([128, D], F32, tag="osb", name=f"ob{nt}")
                nc.scalar.activation(o_sb.rearrange("p (c f) -> p c f", c=2),
                                     o_ps, Copy)
                nc.sync.dma_start(
                    out=out_part[nt * 128:nt * 128 + 128, :], in_=o_sb)

            for q4 in range(NQ4):
                state[("y", q4)] = pp.tile(
                    [65, HPC, 512], F32, tag="y", bufs=1, name=f"y_ps{q4}")
                qk_round(q4, 0)
                for r in range(1, n_rounds):
                    qk_round(q4, r)
                    av_round(q4, r - 1)
                    if q4 > 0:
                        # previous quarter's tail + O-proj, spread so PE
                        # never waits on the recip DMA chain
                        if r == 2:
                            tail_a(q4 - 1)
                        elif r in (8, 10):
                            tail_b(q4 - 1, (r - 8) // 2)
                        elif r in (14, 18, 22, 26):
                            oproj(q4 - 1, (r - 14) // 4)
                av_round(q4, n_rounds - 1)
            tail_a(NQ4 - 1)
            warm2 = pp.tile([128, 2, 512], F32, tag="s", bufs=2, name="warm2")
            for w in range(30):
                nc.tensor.matmul(warm2[:, 0, :], identb_t, kT_all[:, 0, 0:512],
                                 start=(w == 0), stop=(w == 29))
            tail_b(NQ4 - 1, 0)
            tail_b(NQ4 - 1, 1)
            for j in range(4):
                oproj(NQ4 - 1, j)

    return nc


def _ensure_ntff_hook():
    """Register the axon NTFF profiling hook if the agent image lacks
    antenv.axon_hooks (profiling only; kernel runs fine without)."""
    try:
        from antenv.axon_hooks import get_axon_ntff_profile_hook  # noqa: F401
        return
    except ImportError:
        pass
    import types
    import antenv
    from trn_agent_boot.trn_boot import _ntff_profile_via_ctypes

    mod = types.ModuleType("antenv.axon_hooks")
    holder = {}
    mod.set_axon_ntff_profile_hook = lambda h: holder.__setitem__("h", h)
    mod.get_axon_ntff_profile_hook = lambda: holder.get("h")
    sys.modules["antenv.axon_hooks"] = mod
    antenv.axon_hooks = mod
    mod.set_axon_ntff_profile_hook(
        _ntff_profile_via_ctypes("/opt/axon/libaxon_pjrt.so"))


_NC_CACHE: dict = {}


def _get_nc() -> bass.Bass:
    if "nc" not in _NC_CACHE:
        _NC_CACHE["nc"] = _build_nc()
    return _NC_CACHE["nc"]


def kernel(x, alpha, bias, Wq, bq, Wk, bk, Wv, bv, Wo, bo, trace=False):
    bf = ml_dtypes.bfloat16
    x = np.asarray(x, np.float32)
    alpha = np.asarray(alpha, np.float32)
    bias = np.asarray(bias, np.float32)
    Wq = np.asarray(Wq, np.float32); bq = np.asarray(bq, np.float32)
    Wk = np.asarray(Wk, np.float32); bk = np.asarray(bk, np.float32)
    Wv = np.asarray(Wv, np.float32); bv = np.asarray(bv, np.float32)
    Wo = np.asarray(Wo, np.float32); bo = np.asarray(bo, np.float32)

    c = np.ascontiguousarray

    in_maps = []
    per_b = {}
    for b in range(B):
        s = 1.0 + alpha[b]                             # (N,)
        xt = x[b].T.astype(bf)                         # (D, N)
        xkt = (x[b] * s[:, None]).T.astype(bf)
        per_b[b] = {
            # SBUF layout [p, block, t, 512] -> contiguous 8KB block rows
            "xT": c(xt.reshape(8, 128, 4, 512).transpose(1, 2, 0, 3)),
            "xkT": c(xkt.reshape(8, 128, 4, 512).transpose(1, 2, 0, 3)),
            "biasT": c(bias[b].T.astype(bf)),          # (N, N) [m, n]
            "srow": s.reshape(1, N).astype(bf),
        }
    for core in range(NCORES):
        b, hg = divmod(core, 4)
        dsl = slice(hg * DSL, hg * DSL + DSL)
        in_maps.append({
            **per_b[b],
            "wq": c(Wq[:, dsl].astype(bf).reshape(8, 128, DSL).transpose(1, 0, 2)),
            "wk": c(Wk[:, dsl].astype(bf).reshape(8, 128, DSL).transpose(1, 0, 2)),
            "wv": c(Wv[:, dsl].astype(bf).reshape(8, 128, DSL).transpose(1, 0, 2)),
            "wo": c(Wo[dsl, :].astype(bf).reshape(2, 128, D).transpose(1, 0, 2)),
            "bq_r": c(bq[dsl].reshape(1, DSL).astype(bf)),
            "bk_r": c(bk[dsl].reshape(1, DSL).astype(bf)),
            "bv_r": c(bv[dsl].reshape(1, DSL).astype(bf)),
            "onesrow": np.ones((1, 512), bf),
            "ones64": np.ones((65, 64), np.float32),
            "onescol": np.ones((128, 1), bf),
            "identb": np.eye(128, dtype=bf),
        })

    if trace:
        _ensure_ntff_hook()
    nc = _get_nc()
    res = run_bass_kernel_spmd(
        nc, in_maps, core_ids=list(range(NCORES)), trace=trace)

    out = np.zeros((B, N, D), np.float32)
    for core in range(NCORES):
        out[core // 4] += res.results[core]["out_part"]
    out += bo[None, None, :]
    if trace:
        kernel.last_exec_time_ns = res.exec_time_ns
        kernel.last_profile = res.profile_json
    return out
